# revision 1
# baseline (speedup 1.0000x reference)
"""Trainium2 Bass kernel for nn_Cross_Attention (triplet-pool cross-attention gating).

Math (per sample b):
  pools:  Shw[h,w]=max_c x,  Sch[c,h]=max_w x,  Scw[c,w]=max_h x
  3 branches of flat-softmax cross attention between pools -> y12,y13 [h,w],
  y21,y23 [c,h], y31,y32 [c,w]
  training-mode BatchNorm over the *global* batch (cross-core allreduce of
  sum/sumsq), sigmoid gates, and finally
  out = x * (g12*g13)[h,w] * (g21*g23)[c,h] * (g31*g32)[c,w] + x
      = x * (1 + A[h,w]*B[c,h]*Cg[c,w])

Sharding: batch-parallel, 2 samples per core on 8 cores; only the BN batch
stats cross cores (AllReduce of a [128,20] tile).
"""

import numpy as np

import concourse.bacc as bacc
import concourse.mybir as mybir
import concourse.tile as tile
from concourse import masks

f32 = mybir.dt.float32
Alu = mybir.AluOpType
Act = mybir.ActivationFunctionType
X = mybir.AxisListType.X

NCORES = 8
S = 2          # samples per core
C, H, W = 256, 128, 128
CT = 2         # c tiles of 128
HC = 16        # h rows per pass-1 chunk
NCH = H // HC  # 8
CG = 16        # channels per pass-2 group
NG = C // CG   # 16
NP = 20        # bnp columns
EPS = 1e-5


def build_bass(n_cores: int):
    nc = bacc.Bacc("TRN2", target_bir_lowering=False, debug=False,
                   num_devices=n_cores)
    nb_tot = n_cores * S
    n1 = float(nb_tot * H * W)   # bn1 count
    ncn = float(nb_tot * H)      # bnc count (per channel)

    xs = nc.dram_tensor("xs", [S, C, H, W], f32, kind="ExternalInput").ap()
    bn1w = nc.dram_tensor("bn1_w", [1], f32, kind="ExternalInput").ap()
    bn1b = nc.dram_tensor("bn1_b", [1], f32, kind="ExternalInput").ap()
    bncw = nc.dram_tensor("bnc_w", [C], f32, kind="ExternalInput").ap()
    bncb = nc.dram_tensor("bnc_b", [C], f32, kind="ExternalInput").ap()
    outy = nc.dram_tensor("outy", [S, C, H, W], f32, kind="ExternalOutput").ap()

    ccin = nc.dram_tensor("ccin", [128, NP], f32).ap()
    ccout = nc.dram_tensor(
        "ccout", [128, NP], f32,
        addr_space="Shared" if n_cores > 1 else "Local").ap()
    bgd = nc.dram_tensor("bgd", [S, C, H], f32).ap()
    cgd = nc.dram_tensor("cgd", [S, C, W], f32).ap()

    with tile.TileContext(nc) as tc:
        _emit(nc, tc, n_cores, n1, ncn,
              xs, bn1w, bn1b, bncw, bncb, outy, ccin, ccout, bgd, cgd)
    nc.compile()
    return nc


def _emit(nc, tc, n_cores, n1, ncn,
          xs, bn1w, bn1b, bncw, bncb, outy, ccin, ccout, bgd, cgd):
    import contextlib
    stack = contextlib.ExitStack()
    with stack:
        persist = stack.enter_context(tc.tile_pool(name="persist", bufs=1))
        maps = stack.enter_context(tc.tile_pool(name="maps", bufs=2))
        cols = stack.enter_context(tc.tile_pool(name="cols", bufs=4))
        keep = stack.enter_context(tc.tile_pool(name="keep", bufs=1))
        gscr = stack.enter_context(tc.tile_pool(name="gscr", bufs=4))

        # --- setup ---
        identity = persist.tile([128, 128], f32)
        masks.make_identity(nc, identity[:])
        ones_r = persist.tile([1, 128], f32)
        nc.vector.memset(ones_r[:], 1.0)
        ones_c = persist.tile([128, 1], f32)
        nc.vector.memset(ones_c[:], 1.0)
        eps_col = persist.tile([128, 1], f32)
        nc.vector.memset(eps_col[:], EPS)
        wc2 = persist.tile([128, 2], f32)
        nc.sync.dma_start(wc2[:], bncw.rearrange("(t c) -> c t", c=128))
        bc2 = persist.tile([128, 2], f32)
        nc.sync.dma_start(bc2[:], bncb.rearrange("(t c) -> c t", c=128))
        bn1w_sb = persist.tile([1, 1], f32)
        nc.sync.dma_start(bn1w_sb[:], bn1w.unsqueeze(1))
        bn1b_sb = persist.tile([1, 1], f32)
        nc.sync.dma_start(bn1b_sb[:], bn1b.unsqueeze(1))
        wc8 = persist.tile([128, 8], f32)
        bc8 = persist.tile([128, 8], f32)
        for m in range(4):
            nc.vector.tensor_copy(wc8[:, m * 2:m * 2 + 2], wc2[:])
            nc.vector.tensor_copy(bc8[:, m * 2:m * 2 + 2], bc2[:])
        bnp = persist.tile([128, NP], f32)
        nc.vector.memset(bnp[:], 0.0)

        # per-sample persistent maps (bufs=2 -> one slot per sample)
        def smap(name, shape, bufs=None):
            return [maps.tile(shape, f32, name=f"{name}{s}", tag=name,
                              bufs=bufs)
                    for s in range(S)]

        xch = smap("xch", [128, CT * H])    # [c_loc, (t,h)]
        xcw = smap("xcw", [128, CT * W])    # [c_loc, (t,w)]
        xhwT = smap("xhwT", [128, H])       # [w, h]
        shw = smap("shw", [128, W])         # [h, w]
        e12 = smap("e12", [128, C], bufs=1)         # [w, c]
        e12t = smap("e12t", [128, CT * W], bufs=1)  # [c_loc, (t,w)]
        e13 = smap("e13", [128, C], bufs=1)         # [h, c]
        e13t = smap("e13t", [128, CT * H], bufs=1)  # [c_loc, (t,h)]
        e23 = smap("e23", [128, W], bufs=1)         # [h, w]
        e23t = smap("e23t", [128, H], bufs=1)       # [w, h]
        y12T = smap("y12T", [128, H], bufs=1)       # [w, h]
        y12 = smap("y12", [128, W])         # [h, w]
        y13 = smap("y13", [128, W])         # [h, w]
        y21 = smap("y21", [128, CT * H])    # [c_loc, (t,h)]
        y23 = smap("y23", [128, CT * H])
        y31 = smap("y31", [128, CT * W])    # [c_loc, (t,w)]
        y32 = smap("y32", [128, CT * W])
        agate = smap("agate", [128, W])     # [h, w]
        itc = {}   # invT cols [128,1] per (s, branch)
        it1 = {}   # invT [1,1] per (s, branch)

        px2 = stack.enter_context(tc.tile_pool(name="px2", bufs=7))
        pfl = stack.enter_context(tc.tile_pool(name="pfl", bufs=2))
        pg = stack.enter_context(tc.tile_pool(name="pg", bufs=3))

        ps_stack = contextlib.ExitStack()
        with ps_stack:
            px = ps_stack.enter_context(tc.tile_pool(name="px", bufs=3))
            pm = ps_stack.enter_context(tc.tile_pool(name="pm", bufs=3))
            pxcw = ps_stack.enter_context(tc.tile_pool(name="pxcw", bufs=2))
            ps_t = ps_stack.enter_context(
                tc.tile_pool(name="ps_t", bufs=2, space="PSUM"))
            ps_mm = ps_stack.enter_context(
                tc.tile_pool(name="ps_mm", bufs=2, space="PSUM"))
            ps_ty = ps_stack.enter_context(
                tc.tile_pool(name="ps_ty", bufs=2, space="PSUM"))

            # ---------------- pass 1: pooled descriptors ----------------
            for s in range(S):
                xcwp = [pxcw.tile([128, NCH * W], f32, name=f"xcwp{s}{t}",
                                  tag="xcwp") for t in range(CT)]
                for k in range(NCH):
                    xts = []
                    for t in range(CT):
                        xt = px.tile([128, HC, W], f32, name=f"xt{s}{t}{k}",
                                     tag="xt")
                        nc.sync.dma_start(
                            xt[:], xs[s, t * 128:(t + 1) * 128,
                                      k * HC:(k + 1) * HC, :])
                        xts.append(xt)
                        # x_ch partial: max over w
                        nc.vector.tensor_reduce(
                            out=xch[s][:, t * H + k * HC: t * H + (k + 1) * HC],
                            in_=xt[:], axis=X, op=Alu.max)
                        # x_cw partial: max over h-sub
                        nc.vector.tensor_reduce(
                            out=xcwp[t][:, k * W:(k + 1) * W],
                            in_=xt[:].transpose([0, 2, 1]), axis=X, op=Alu.max)
                    # fold the two c tiles for x_hw
                    mch = pm.tile([128, HC, W], f32, name=f"m{s}{k}", tag="m")
                    nc.vector.tensor_tensor(
                        out=mch[:], in0=xts[0][:], in1=xts[1][:], op=Alu.max)
                    for g8 in range(HC // 8):
                        tr = ps_t.tile([128, 8, 128], f32, name=f"tr{s}{k}{g8}",
                                       tag="tr")
                        for j in range(8):
                            nc.tensor.transpose(
                                tr[:, j, :], mch[:, g8 * 8 + j, :], identity[:])
                        nc.vector.tensor_reduce(
                            out=xhwT[s][:, k * HC + g8 * 8: k * HC + g8 * 8 + 8],
                            in_=tr[:], axis=X, op=Alu.max)
                for t in range(CT):
                    nc.vector.tensor_reduce(
                        out=xcw[s][:, t * W:(t + 1) * W],
                        in_=xcwp[t][:].rearrange("p (k w) -> p w k", w=W),
                        axis=X, op=Alu.max)

            # ---------------- phase B: attention ----------------
            def psum_copy_to(dst, src_ps):
                nc.scalar.copy(dst, src_ps)

            def transpose_to(dst, src_sb, nblk, name):
                # src [128, nblk*128] -> dst [128, nblk*128] blockwise T
                for t in range(nblk):
                    tp = ps_mm.tile([128, 128], f32, name=f"tp{name}{t}",
                                    tag="mm")
                    nc.tensor.transpose(
                        tp[:], src_sb[:, t * 128:(t + 1) * 128], identity[:])
                    psum_copy_to(dst[:, t * 128:(t + 1) * 128], tp[:])

            def softmax(s, br, sim_ps, ncol, e_dst):
                rowmax = cols.tile([128, 1], f32, name=f"rm{s}{br}", tag="c1")
                nc.vector.tensor_reduce(out=rowmax[:], in_=sim_ps[:], axis=X,
                                        op=Alu.max)
                rmt = ps_ty.tile([1, 128], f32, name=f"rmt{s}{br}", tag="ty")
                nc.tensor.transpose(rmt[:], rowmax[:], identity[:])
                gmax = cols.tile([1, 1], f32, name=f"gm{s}{br}", tag="c0")
                nc.vector.tensor_reduce(out=gmax[:], in_=rmt[:], axis=X,
                                        op=Alu.max)
                ngmax = cols.tile([1, 1], f32, name=f"ngm{s}{br}", tag="c0")
                nc.scalar.mul(ngmax[:], gmax[:], -1.0)
                nm_ps = ps_ty.tile([128, 1], f32, name=f"nmp{s}{br}", tag="ty")
                nc.tensor.matmul(nm_ps[:], ones_r[:], ngmax[:])
                nmcol = cols.tile([128, 1], f32, name=f"nmc{s}{br}", tag="c1")
                psum_copy_to(nmcol[:], nm_ps[:])
                rowsum = cols.tile([128, 1], f32, name=f"rs{s}{br}", tag="c1")
                nc.scalar.activation(out=e_dst[:], in_=sim_ps[:], func=Act.Exp,
                                     bias=nmcol[:], scale=1.0,
                                     accum_out=rowsum[:])
                tot_ps = ps_ty.tile([1, 1], f32, name=f"tot{s}{br}", tag="ty")
                nc.tensor.matmul(tot_ps[:], rowsum[:], ones_c[:])
                invt = keep.tile([1, 1], f32, name=f"it{s}{br}",
                                 tag=f"it{s}{br}")
                nc.vector.reciprocal(invt[:], tot_ps[:])
                ic_ps = ps_ty.tile([128, 1], f32, name=f"icp{s}{br}", tag="ty")
                nc.tensor.matmul(ic_ps[:], ones_r[:], invt[:])
                iccol = keep.tile([128, 1], f32, name=f"icc{s}{br}",
                                  tag=f"icc{s}{br}")
                psum_copy_to(iccol[:], ic_ps[:])
                it1[(s, br)] = invt
                itc[(s, br)] = iccol

            scht = smap("scht", [128, CT * H], bufs=1)  # [h, (t,c_loc)] -> x_ch^T
            scwt = smap("scwt", [128, CT * W], bufs=1)  # [w, (t,c_loc)] -> x_cw^T

            for s in range(S):
                transpose_to(scht[s], xch[s], CT, f"sch{s}")
                transpose_to(scwt[s], xcw[s], CT, f"scw{s}")
                shp = ps_mm.tile([128, 128], f32, name=f"shp{s}", tag="mm")
                nc.tensor.transpose(shp[:], xhwT[s][:], identity[:])
                psum_copy_to(shw[s][:], shp[:])

                # --- branch 12: sim12[w,c] = sum_h Shw[h,w] Sch[c,h]
                sim12 = ps_mm.tile([128, C], f32, name=f"s12_{s}", tag="mm")
                nc.tensor.matmul(sim12[:], shw[s][:], scht[s][:])
                softmax(s, 12, sim12, C, e12[s])
                transpose_to(e12t[s], e12[s], CT, f"e12{s}")
                # y12T[w,h] = sum_c e12t[c,w]^T ... accumulate 2 c tiles
                y12p = ps_mm.tile([128, H], f32, name=f"y12p{s}", tag="mm")
                for t in range(CT):
                    nc.tensor.matmul(
                        y12p[:], e12t[s][:, t * W:(t + 1) * W],
                        xch[s][:, t * H:(t + 1) * H],
                        start=(t == 0), stop=(t == CT - 1))
                psum_copy_to(y12T[s][:], y12p[:])
                # y21[c,h] per c tile
                for t in range(CT):
                    y21p = ps_mm.tile([128, H], f32, name=f"y21p{s}{t}",
                                      tag="mm")
                    nc.tensor.matmul(y21p[:], e12[s][:, t * 128:(t + 1) * 128],
                                     xhwT[s][:])
                    psum_copy_to(y21[s][:, t * H:(t + 1) * H], y21p[:])

                # --- branch 13: sim13[h,c] = sum_w Shw[h,w] Scw[c,w]
                sim13 = ps_mm.tile([128, C], f32, name=f"s13_{s}", tag="mm")
                nc.tensor.matmul(sim13[:], xhwT[s][:], scwt[s][:])
                softmax(s, 13, sim13, C, e13[s])
                transpose_to(e13t[s], e13[s], CT, f"e13{s}")
                y13p = ps_mm.tile([128, W], f32, name=f"y13p{s}", tag="mm")
                for t in range(CT):
                    nc.tensor.matmul(
                        y13p[:], e13t[s][:, t * H:(t + 1) * H],
                        xcw[s][:, t * W:(t + 1) * W],
                        start=(t == 0), stop=(t == CT - 1))
                psum_copy_to(y13[s][:], y13p[:])
                for t in range(CT):
                    y31p = ps_mm.tile([128, W], f32, name=f"y31p{s}{t}",
                                      tag="mm")
                    nc.tensor.matmul(y31p[:], e13[s][:, t * 128:(t + 1) * 128],
                                     shw[s][:])
                    psum_copy_to(y31[s][:, t * W:(t + 1) * W], y31p[:])

                # --- branch 23: sim23[h,w] = sum_c Sch[c,h] Scw[c,w]
                sim23 = ps_mm.tile([128, W], f32, name=f"s23_{s}", tag="mm")
                for t in range(CT):
                    nc.tensor.matmul(
                        sim23[:], xch[s][:, t * H:(t + 1) * H],
                        xcw[s][:, t * W:(t + 1) * W],
                        start=(t == 0), stop=(t == CT - 1))
                softmax(s, 23, sim23, W, e23[s])
                transpose_to(e23t[s], e23[s], 1, f"e23{s}")
                for t in range(CT):
                    y23p = ps_mm.tile([128, H], f32, name=f"y23p{s}{t}",
                                      tag="mm")
                    nc.tensor.matmul(y23p[:], scwt[s][:, t * W:(t + 1) * W],
                                     e23t[s][:])
                    psum_copy_to(y23[s][:, t * H:(t + 1) * H], y23p[:])
                    y32p = ps_mm.tile([128, W], f32, name=f"y32p{s}{t}",
                                      tag="mm")
                    nc.tensor.matmul(y32p[:], scht[s][:, t * H:(t + 1) * H],
                                     e23[s][:])
                    psum_copy_to(y32[s][:, t * W:(t + 1) * W], y32p[:])

                # y12 = transpose(y12T)
                y12pp = ps_mm.tile([128, 128], f32, name=f"y12pp{s}", tag="mm")
                nc.tensor.transpose(y12pp[:], y12T[s][:], identity[:])
                psum_copy_to(y12[s][:], y12pp[:])

            # ---------------- phase C: BN partials ----------------
            ysq = gscr.tile([128, 128], f32, name="ysq", tag="ysq", bufs=2)
            for s in range(S):
                it2 = {}
                for br in (12, 13, 23):
                    t2 = keep.tile([128, 1], f32, name=f"it2_{s}{br}",
                                   tag=f"it2_{s}{br}")
                    nc.vector.tensor_tensor(out=t2[:], in0=itc[(s, br)][:],
                                            in1=itc[(s, br)][:], op=Alu.mult)
                    it2[br] = t2
                bnc_maps = [(0, y21[s], 12), (1, y23[s], 23),
                            (2, y31[s], 13), (3, y32[s], 23)]
                for m, ysb, br in bnc_maps:
                    r2 = cols.tile([128, 2], f32, name=f"r{s}{m}", tag="c2")
                    nc.vector.tensor_reduce(
                        out=r2[:], in_=ysb[:].rearrange("p (t h) -> p t h",
                                                        t=CT),
                        axis=X, op=Alu.add)
                    nc.vector.scalar_tensor_tensor(
                        out=bnp[:, m * 2:m * 2 + 2], in0=r2[:],
                        scalar=itc[(s, br)][:], in1=bnp[:, m * 2:m * 2 + 2],
                        op0=Alu.mult, op1=Alu.add)
                    for t in range(CT):
                        col = m * 2 + t
                        blk = ysb[:, t * 128:(t + 1) * 128]
                        sq = cols.tile([128, 1], f32, name=f"sq{s}{m}{t}",
                                       tag="c1")
                        nc.scalar.activation(out=ysq[:], in_=blk,
                                             func=Act.Square, accum_out=sq[:])
                        nc.vector.scalar_tensor_tensor(
                            out=bnp[:, 8 + col:9 + col], in0=sq[:],
                            scalar=it2[br][:], in1=bnp[:, 8 + col:9 + col],
                            op0=Alu.mult, op1=Alu.add)
                # bn1 partials (partition 0, cols 16..19)
                for j, (ymap, br) in enumerate(((y12T[s], 12), (y13[s], 13))):
                    i1 = it1[(s, br)]
                    i2 = cols.tile([1, 1], f32, name=f"i2_{s}{j}", tag="c0")
                    nc.vector.tensor_tensor(out=i2[:], in0=i1[:], in1=i1[:],
                                            op=Alu.mult)
                    rs = cols.tile([128, 1], f32, name=f"rs1_{s}{j}", tag="c1")
                    nc.vector.tensor_reduce(out=rs[:], in_=ymap[:], axis=X,
                                            op=Alu.add)
                    tp = ps_ty.tile([1, 1], f32, name=f"t1_{s}{j}", tag="ty")
                    nc.tensor.matmul(tp[:], rs[:], ones_c[:])
                    nc.vector.scalar_tensor_tensor(
                        out=bnp[0:1, 16 + 2 * j:17 + 2 * j], in0=tp[:],
                        scalar=i1[:], in1=bnp[0:1, 16 + 2 * j:17 + 2 * j],
                        op0=Alu.mult, op1=Alu.add)
                    sqc = cols.tile([128, 1], f32, name=f"sqc{s}{j}", tag="c1")
                    nc.scalar.activation(out=ysq[:], in_=ymap[:],
                                         func=Act.Square, accum_out=sqc[:])
                    tp2 = ps_ty.tile([1, 1], f32, name=f"t2_{s}{j}", tag="ty")
                    nc.tensor.matmul(tp2[:], sqc[:], ones_c[:])
                    nc.vector.scalar_tensor_tensor(
                        out=bnp[0:1, 17 + 2 * j:18 + 2 * j], in0=tp2[:],
                        scalar=i2[:], in1=bnp[0:1, 17 + 2 * j:18 + 2 * j],
                        op0=Alu.mult, op1=Alu.add)

            # ---------------- allreduce ----------------
            nc.sync.dma_start(ccin, bnp[:])
            if n_cores > 1:
                nc.gpsimd.collective_compute(
                    "AllReduce", Alu.add,
                    replica_groups=[list(range(n_cores))],
                    ins=[ccin], outs=[ccout])
            else:
                nc.sync.dma_start(ccout, ccin)
            bnpg = persist.tile([128, NP], f32)
            nc.sync.dma_start(bnpg[:], ccout)

            # ---------------- phase D: BN finalize + gates ----------------
            sm = persist.tile([128, 8], f32, name="mu8")
            nc.vector.tensor_scalar_mul(sm[:], bnpg[:, 0:8], 1.0 / ncn)
            m2 = persist.tile([128, 8], f32, name="m28")
            nc.vector.tensor_tensor(out=m2[:], in0=sm[:], in1=sm[:],
                                    op=Alu.mult)
            var8 = persist.tile([128, 8], f32, name="var8")
            nc.vector.scalar_tensor_tensor(
                out=var8[:], in0=bnpg[:, 8:16], scalar=1.0 / ncn, in1=m2[:],
                op0=Alu.mult, op1=Alu.subtract)
            sd8 = persist.tile([128, 8], f32, name="sd8")
            nc.scalar.activation(out=sd8[:], in_=var8[:], func=Act.Sqrt,
                                 bias=eps_col[:])
            rstd8 = persist.tile([128, 8], f32, name="rstd8")
            nc.vector.reciprocal(rstd8[:], sd8[:])
            scale8 = persist.tile([128, 8], f32, name="scale8")
            nc.vector.tensor_tensor(out=scale8[:], in0=rstd8[:], in1=wc8[:],
                                    op=Alu.mult)
            q8 = persist.tile([128, 8], f32, name="q8")
            nc.vector.tensor_tensor(out=q8[:], in0=sm[:], in1=scale8[:],
                                    op=Alu.mult)
            shift8 = persist.tile([128, 8], f32, name="shift8")
            nc.vector.scalar_tensor_tensor(
                out=shift8[:], in0=q8[:], scalar=-1.0, in1=bc8[:],
                op0=Alu.mult, op1=Alu.add)

            # bn1 scalars on partition 0 (j=0 -> y12, j=1 -> y13)
            sc1 = []
            sh1col = []
            for j in range(2):
                mu1 = cols.tile([1, 1], f32, name=f"mu1_{j}", tag="c0")
                nc.vector.tensor_scalar_mul(mu1[:], bnpg[0:1, 16 + 2 * j:17 + 2 * j],
                                            1.0 / n1)
                m21 = cols.tile([1, 1], f32, name=f"m21_{j}", tag="c0")
                nc.vector.tensor_tensor(out=m21[:], in0=mu1[:], in1=mu1[:],
                                        op=Alu.mult)
                v1 = cols.tile([1, 1], f32, name=f"v1_{j}", tag="c0")
                nc.vector.scalar_tensor_tensor(
                    out=v1[:], in0=bnpg[0:1, 17 + 2 * j:18 + 2 * j],
                    scalar=1.0 / n1, in1=m21[:], op0=Alu.mult,
                    op1=Alu.subtract)
                sd1 = cols.tile([1, 1], f32, name=f"sd1_{j}", tag="c0")
                nc.scalar.activation(out=sd1[:], in_=v1[:], func=Act.Sqrt,
                                     bias=eps_col[0:1, :])
                rst1 = cols.tile([1, 1], f32, name=f"rst1_{j}", tag="c0")
                nc.vector.reciprocal(rst1[:], sd1[:])
                sc = keep.tile([1, 1], f32, name=f"sc1_{j}",
                               tag=f"sc1_{j}")
                nc.vector.tensor_tensor(out=sc[:], in0=rst1[:], in1=bn1w_sb[:],
                                        op=Alu.mult)
                sc1.append(sc)
                q1 = cols.tile([1, 1], f32, name=f"q1_{j}", tag="c0")
                nc.vector.tensor_tensor(out=q1[:], in0=mu1[:], in1=sc[:],
                                        op=Alu.mult)
                sh = cols.tile([1, 1], f32, name=f"sh1_{j}", tag="c0")
                nc.vector.scalar_tensor_tensor(
                    out=sh[:], in0=q1[:], scalar=-1.0, in1=bn1b_sb[:],
                    op0=Alu.mult, op1=Alu.add)
                shp_ = ps_ty.tile([128, 1], f32, name=f"shp1_{j}", tag="ty")
                nc.tensor.matmul(shp_[:], ones_r[:], sh[:])
                shcol = keep.tile([128, 1], f32, name=f"shc1_{j}",
                                  tag=f"shc1_{j}")
                psum_copy_to(shcol[:], shp_[:])
                sh1col.append(shcol)

            bgate = smap("bgate", [128, CT * H])
            cgate = smap("cgate", [128, CT * W])
            for s in range(S):
                # A gate
                g1 = gscr.tile([128, W], f32, name=f"g12_{s}", tag="ga")
                g2 = gscr.tile([128, W], f32, name=f"g13_{s}", tag="ga")
                for j, (ymap, br, g) in enumerate(
                        ((y12[s], 12, g1), (y13[s], 13, g2))):
                    scs = cols.tile([1, 1], f32, name=f"scs{s}{j}", tag="c0")
                    nc.vector.tensor_tensor(out=scs[:], in0=sc1[j][:],
                                            in1=it1[(s, br)][:], op=Alu.mult)
                    scp = ps_ty.tile([128, 1], f32, name=f"scp{s}{j}",
                                     tag="ty")
                    nc.tensor.matmul(scp[:], ones_r[:], scs[:])
                    sccol = cols.tile([128, 1], f32, name=f"sccol{s}{j}",
                                      tag="c1")
                    psum_copy_to(sccol[:], scp[:])
                    nc.scalar.activation(out=g[:], in_=ymap[:],
                                         func=Act.Sigmoid, bias=sh1col[j][:],
                                         scale=sccol[:])
                nc.vector.tensor_tensor(out=agate[s][:], in0=g1[:], in1=g2[:],
                                        op=Alu.mult)
                # B / C gates
                for gate, (ma, bra), (mb, brb), ysa, ysb_ in (
                        (bgate[s], (0, 12), (1, 23), y21[s], y23[s]),
                        (cgate[s], (2, 13), (3, 23), y31[s], y32[s])):
                    ga = gscr.tile([128, CT * 128], f32, name=f"ga{s}{ma}",
                                   tag="gb")
                    gb = gscr.tile([128, CT * 128], f32, name=f"gb{s}{mb}",
                                   tag="gb")
                    for (m, br, ysrc, gdst) in ((ma, bra, ysa, ga),
                                                (mb, brb, ysb_, gb)):
                        for t in range(CT):
                            col = m * 2 + t
                            scc = cols.tile([128, 1], f32,
                                            name=f"scc{s}{m}{t}", tag="c1")
                            nc.vector.tensor_tensor(
                                out=scc[:], in0=scale8[:, col:col + 1],
                                in1=itc[(s, br)][:], op=Alu.mult)
                            nc.scalar.activation(
                                out=gdst[:, t * 128:(t + 1) * 128],
                                in_=ysrc[:, t * 128:(t + 1) * 128],
                                func=Act.Sigmoid,
                                bias=shift8[:, col:col + 1], scale=scc[:])
                    nc.vector.tensor_tensor(out=gate[:], in0=ga[:], in1=gb[:],
                                            op=Alu.mult)
                # dump B/C gates to dram for flat-row staging
                nc.scalar.dma_start(
                    bgd[s].rearrange("(t c) h -> c t h", t=CT),
                    bgate[s][:].rearrange("p (t h) -> p t h", t=CT))
                nc.scalar.dma_start(
                    cgd[s].rearrange("(t c) w -> c t w", t=CT),
                    cgate[s][:].rearrange("p (t w) -> p t w", t=CT))

        # ---------------- phase E: apply ----------------
        e_stack = contextlib.ExitStack()
        with e_stack:
            ps_o = e_stack.enter_context(
                tc.tile_pool(name="ps_o", bufs=2, space="PSUM"))
            for s in range(S):
                for g in range(NG):
                    c0 = g * CG
                    fl = pfl.tile([1, 2 * CG * H], f32, name=f"fl{s}{g}",
                                  tag="fl")
                    bfl = fl[:, 0:CG * H]
                    cfl = fl[:, CG * H:2 * CG * H]
                    nc.sync.dma_start(
                        bfl, bgd[s, c0:c0 + CG, :]
                        .rearrange("c h -> (c h)").unsqueeze(0))
                    nc.sync.dma_start(
                        cfl, cgd[s, c0:c0 + CG, :]
                        .rearrange("c w -> (c w)").unsqueeze(0))
                    ops = ps_o.tile([128, CG, W], f32, name=f"o{s}{g}",
                                    tag="o")
                    for i in range(CG):
                        nc.tensor.matmul(
                            ops[:, i, :], bfl[:, i * H:(i + 1) * H],
                            cfl[:, i * W:(i + 1) * W])
                    x16 = px2.tile([128, CG, W], f32, name=f"x16_{s}{g}",
                                   tag="x16")
                    nc.sync.dma_start(
                        x16[:], xs[s, c0:c0 + CG, :, :].transpose([1, 0, 2]))
                    g16 = pg.tile([128, CG, W], f32, name=f"g16_{s}{g}",
                                  tag="g16")
                    av = agate[s][:].unsqueeze(1).broadcast_to([128, CG, W])
                    nc.vector.scalar_tensor_tensor(
                        out=g16[:], in0=ops[:], scalar=1.0, in1=av,
                        op0=Alu.mult, op1=Alu.mult)
                    nc.vector.scalar_tensor_tensor(
                        out=x16[:], in0=g16[:], scalar=1.0, in1=x16[:],
                        op0=Alu.add, op1=Alu.mult)
                    nc.scalar.dma_start(
                        outy[s, c0:c0 + CG, :, :].transpose([1, 0, 2]), x16[:])


_NC_CACHE = {}
LAST_RESULT = None


def _get_nc(n_cores: int):
    if n_cores not in _NC_CACHE:
        _NC_CACHE[n_cores] = build_bass(n_cores)
    return _NC_CACHE[n_cores]


def kernel(**inputs) -> np.ndarray:
    from concourse.bass_utils import run_bass_kernel_spmd

    x = np.ascontiguousarray(inputs["x"], dtype=np.float32)
    bn1_w = np.ascontiguousarray(inputs["bn1_w"], dtype=np.float32)
    bn1_b = np.ascontiguousarray(inputs["bn1_b"], dtype=np.float32)
    bnc_w = np.ascontiguousarray(inputs["bnc_w"], dtype=np.float32)
    bnc_b = np.ascontiguousarray(inputs["bnc_b"], dtype=np.float32)
    B = x.shape[0]
    assert B == NCORES * S, (B, NCORES, S)

    nc = _get_nc(NCORES)
    in_maps = []
    for i in range(NCORES):
        in_maps.append({
            "xs": np.ascontiguousarray(x[i * S:(i + 1) * S]),
            "bn1_w": bn1_w, "bn1_b": bn1_b,
            "bnc_w": bnc_w, "bnc_b": bnc_b,
        })
    res = run_bass_kernel_spmd(nc, in_maps, core_ids=list(range(NCORES)))
    global LAST_RESULT
    LAST_RESULT = res
    out = np.concatenate([res.results[i]["outy"] for i in range(NCORES)],
                         axis=0)
    return out



# revision 16
# speedup vs baseline: 1.2726x; 1.2726x over previous
"""Trainium2 Bass kernel for nn_Cross_Attention (triplet-pool cross-attention gating).

Math (per sample b):
  pools:  Shw[h,w]=max_c x,  Sch[c,h]=max_w x,  Scw[c,w]=max_h x
  3 branches of flat-softmax cross attention between pools -> y12,y13 [h,w],
  y21,y23 [c,h], y31,y32 [c,w]
  training-mode BatchNorm over the *global* batch (cross-core allreduce of
  sum/sumsq), sigmoid gates, and finally
  out = x * (g12*g13)[h,w] * (g21*g23)[c,h] * (g31*g32)[c,w] + x
      = x * (1 + A[h,w]*B[c,h]*Cg[c,w])

Sharding: batch-parallel, 2 samples per core on 8 cores; only the BN batch
stats cross cores (AllReduce of a [128,20] tile).

Apply phase keeps partition=channel so both the x reload and the out store
are contiguous 8KB-per-partition DMAs; A[h,w] is replicated across the 128
channel partitions via PE row-broadcasts (bf16), and the first KRES h-chunks
of x are kept resident in SBUF as bf16 from pass 1 to cut the reload.
"""

import numpy as np

import concourse.bacc as bacc
import concourse.mybir as mybir
import concourse.tile as tile
from concourse import masks

f32 = mybir.dt.float32
bf16 = mybir.dt.bfloat16
Alu = mybir.AluOpType
Act = mybir.ActivationFunctionType
X = mybir.AxisListType.X

NCORES = 8
S = 2          # samples per core
C, H, W = 256, 128, 128
CT = 2         # c tiles of 128
HC = 16        # h rows per chunk
NCH = H // HC  # 8
KRES = 5       # h-chunks per (s,t) kept resident in SBUF as bf16
NP = 20        # bnp columns
EPS = 1e-5


def build_bass(n_cores: int, sync_start: bool = False, phases: str = "ABCDE"):
    """sync_start/phases are for timing probes only: sync_start prepends a
    tiny AllReduce so all cores start main work in lockstep (makes full
    device time visible to the marginal-time harness); phases truncates."""
    nc = bacc.Bacc("TRN2", target_bir_lowering=False, debug=False,
                   num_devices=n_cores)
    nb_tot = n_cores * S
    n1 = float(nb_tot * H * W)   # bn1 count
    ncn = float(nb_tot * H)      # bnc count (per channel)

    xs = nc.dram_tensor("xs", [S, C, H, W], f32, kind="ExternalInput").ap()
    bn1w = nc.dram_tensor("bn1_w", [1], f32, kind="ExternalInput").ap()
    bn1b = nc.dram_tensor("bn1_b", [1], f32, kind="ExternalInput").ap()
    bncw = nc.dram_tensor("bnc_w", [C], f32, kind="ExternalInput").ap()
    bncb = nc.dram_tensor("bnc_b", [C], f32, kind="ExternalInput").ap()
    outy = nc.dram_tensor("outy", [S, C, H, W], f32, kind="ExternalOutput").ap()

    ccin = nc.dram_tensor("ccin", [128, NP], f32).ap()
    ccout = nc.dram_tensor(
        "ccout", [128, NP], f32,
        addr_space="Shared" if n_cores > 1 else "Local").ap()
    adram = nc.dram_tensor("adram", [S, H * W], bf16).ap()
    sync_bufs = None
    if sync_start:
        sin = nc.dram_tensor("sin", [1, 1], f32).ap()
        sout = nc.dram_tensor(
            "sout", [1, 1], f32,
            addr_space="Shared" if n_cores > 1 else "Local").ap()
        sync_bufs = (sin, sout)

    with tile.TileContext(nc) as tc:
        _emit(nc, tc, n_cores, n1, ncn,
              xs, bn1w, bn1b, bncw, bncb, outy, ccin, ccout, adram,
              sync_bufs, phases)
    nc.compile()
    return nc


def _emit(nc, tc, n_cores, n1, ncn,
          xs, bn1w, bn1b, bncw, bncb, outy, ccin, ccout, adram,
          sync_bufs=None, phases="ABCDE"):
    import contextlib
    stack = contextlib.ExitStack()
    with stack:
        persist = stack.enter_context(tc.tile_pool(name="persist", bufs=1))
        maps = stack.enter_context(tc.tile_pool(name="maps", bufs=2))
        cols = stack.enter_context(tc.tile_pool(name="cols", bufs=4))
        keep = stack.enter_context(tc.tile_pool(name="keep", bufs=1))
        gscr = stack.enter_context(tc.tile_pool(name="gscr", bufs=4))

        # --- timing-only start barrier: a tiny AllReduce whose result is
        # loaded on the sync DMA queue, so every later HWDGE load (FIFO per
        # engine) waits until all cores have started this iteration ---
        if sync_bufs is not None:
            sin, sout = sync_bufs
            st0 = persist.tile([1, 1], f32, name="st0")
            nc.vector.memset(st0[:], 1.0)
            nc.sync.dma_start(sin, st0[:])
            nc.gpsimd.collective_compute(
                "AllReduce", Alu.add,
                replica_groups=[list(range(n_cores))],
                ins=[sin], outs=[sout])
            st1 = persist.tile([1, 1], f32, name="st1")
            nc.sync.dma_start(st1[:], sout)

        # --- setup ---
        identity = persist.tile([128, 128], f32)
        masks.make_identity(nc, identity[:])
        ones_r = persist.tile([1, 128], f32)
        nc.vector.memset(ones_r[:], 1.0)
        ones_c = persist.tile([128, 1], f32)
        nc.vector.memset(ones_c[:], 1.0)
        eps_col = persist.tile([128, 1], f32)
        nc.vector.memset(eps_col[:], EPS)
        wc2 = persist.tile([128, 2], f32)
        nc.sync.dma_start(wc2[:], bncw.rearrange("(t c) -> c t", c=128))
        bc2 = persist.tile([128, 2], f32)
        nc.sync.dma_start(bc2[:], bncb.rearrange("(t c) -> c t", c=128))
        bn1w_sb = persist.tile([1, 1], f32)
        nc.sync.dma_start(bn1w_sb[:], bn1w.unsqueeze(1))
        bn1b_sb = persist.tile([1, 1], f32)
        nc.sync.dma_start(bn1b_sb[:], bn1b.unsqueeze(1))
        wc8 = persist.tile([128, 8], f32)
        bc8 = persist.tile([128, 8], f32)
        for m in range(4):
            nc.vector.tensor_copy(wc8[:, m * 2:m * 2 + 2], wc2[:])
            nc.vector.tensor_copy(bc8[:, m * 2:m * 2 + 2], bc2[:])
        bnp = persist.tile([128, NP], f32)
        nc.vector.memset(bnp[:], 0.0)

        # per-sample persistent maps (bufs=2 -> one slot per sample)
        def smap(name, shape, bufs=None, dtype=f32):
            return [maps.tile(shape, dtype, name=f"{name}{s}", tag=name,
                              bufs=bufs)
                    for s in range(S)]

        xch = smap("xch", [128, CT * H])    # [c_loc, (t,h)]
        xcw = smap("xcw", [128, CT * W])    # [c_loc, (t,w)]
        xhwT = smap("xhwT", [128, H])       # [w, h]
        shw = smap("shw", [128, W])         # [h, w]
        e12 = smap("e12", [128, C], bufs=1)         # [w, c]
        e12t = smap("e12t", [128, CT * W], bufs=1)  # [c_loc, (t,w)]
        e13 = smap("e13", [128, C], bufs=1)         # [h, c]
        e13t = smap("e13t", [128, CT * H], bufs=1)  # [c_loc, (t,h)]
        e23 = smap("e23", [128, W], bufs=1)         # [h, w]
        e23t = smap("e23t", [128, H], bufs=1)       # [w, h]
        y12T = smap("y12T", [128, H], bufs=1)       # [w, h]
        y12 = smap("y12", [128, W])         # [h, w]
        y13 = smap("y13", [128, W])         # [h, w]
        y21 = smap("y21", [128, CT * H])    # [c_loc, (t,h)]
        y23 = smap("y23", [128, CT * H])
        y31 = smap("y31", [128, CT * W])    # [c_loc, (t,w)]
        y32 = smap("y32", [128, CT * W])
        agate = smap("agate", [128, W], dtype=bf16)     # [h, w]
        itc = {}   # invT cols [128,1] per (s, branch)
        it1 = {}   # invT [1,1] per (s, branch)

        # resident bf16 x chunks: [c_loc, k, h_sub, w] per (s, t)
        xres = [[persist.tile([128, KRES * HC, W], bf16,
                              name=f"xres{s}{t}", tag=f"xres{s}{t}")
                 for t in range(CT)] for s in range(S)] if KRES else None

        ps_stack = contextlib.ExitStack()
        with ps_stack:
            px = ps_stack.enter_context(tc.tile_pool(name="px", bufs=3))
            pm = ps_stack.enter_context(tc.tile_pool(name="pm", bufs=3))
            pxcw = ps_stack.enter_context(tc.tile_pool(name="pxcw", bufs=2))
            ps_t = ps_stack.enter_context(
                tc.tile_pool(name="ps_t", bufs=2, space="PSUM"))
            ps_mm = ps_stack.enter_context(
                tc.tile_pool(name="ps_mm", bufs=2, space="PSUM"))
            ps_ty = ps_stack.enter_context(
                tc.tile_pool(name="ps_ty", bufs=2, space="PSUM"))

            # ---------------- pass 1: pooled descriptors ----------------
            for s in range(S):
                xcwp = [pxcw.tile([128, NCH * W], f32, name=f"xcwp{s}{t}",
                                  tag="xcwp") for t in range(CT)]
                for k in range(NCH):
                    xts = []
                    for t in range(CT):
                        xt = px.tile([128, HC, W], f32, name=f"xt{s}{t}{k}",
                                     tag="xt")
                        nc.sync.dma_start(
                            xt[:], xs[s, t * 128:(t + 1) * 128,
                                      k * HC:(k + 1) * HC, :])
                        xts.append(xt)
                        # x_ch partial: max over w
                        nc.vector.tensor_reduce(
                            out=xch[s][:, t * H + k * HC: t * H + (k + 1) * HC],
                            in_=xt[:], axis=X, op=Alu.max)
                        # x_cw partial: max over h-sub
                        nc.vector.tensor_reduce(
                            out=xcwp[t][:, k * W:(k + 1) * W],
                            in_=xt[:].transpose([0, 2, 1]), axis=X, op=Alu.max)
                        # resident bf16 copy for the apply phase
                        if k < KRES:
                            nc.scalar.copy(
                                xres[s][t][:, k * HC:(k + 1) * HC, :], xt[:])
                    # fold the two c tiles for x_hw
                    mch = pm.tile([128, HC, W], f32, name=f"m{s}{k}", tag="m")
                    nc.vector.tensor_tensor(
                        out=mch[:], in0=xts[0][:], in1=xts[1][:], op=Alu.max)
                    for g8 in range(HC // 8):
                        tr = ps_t.tile([128, 8, 128], f32, name=f"tr{s}{k}{g8}",
                                       tag="tr")
                        for j in range(8):
                            nc.tensor.transpose(
                                tr[:, j, :], mch[:, g8 * 8 + j, :], identity[:])
                        nc.vector.tensor_reduce(
                            out=xhwT[s][:, k * HC + g8 * 8: k * HC + g8 * 8 + 8],
                            in_=tr[:], axis=X, op=Alu.max)
                for t in range(CT):
                    nc.vector.tensor_reduce(
                        out=xcw[s][:, t * W:(t + 1) * W],
                        in_=xcwp[t][:].rearrange("p (k w) -> p w k", w=W),
                        axis=X, op=Alu.max)

            if "B" not in phases:
                return

            # ---------------- phase B: attention ----------------
            def psum_copy_to(dst, src_ps):
                nc.scalar.copy(dst, src_ps)

            def transpose_to(dst, src_sb, nblk, name):
                # src [128, nblk*128] -> dst [128, nblk*128] blockwise T
                for t in range(nblk):
                    tp = ps_mm.tile([128, 128], f32, name=f"tp{name}{t}",
                                    tag="mm")
                    nc.tensor.transpose(
                        tp[:], src_sb[:, t * 128:(t + 1) * 128], identity[:])
                    psum_copy_to(dst[:, t * 128:(t + 1) * 128], tp[:])

            def softmax(s, br, sim_ps, ncol, e_dst):
                rowmax = cols.tile([128, 1], f32, name=f"rm{s}{br}", tag="c1")
                nc.vector.tensor_reduce(out=rowmax[:], in_=sim_ps[:], axis=X,
                                        op=Alu.max)
                rmt = ps_ty.tile([1, 128], f32, name=f"rmt{s}{br}", tag="ty")
                nc.tensor.transpose(rmt[:], rowmax[:], identity[:])
                gmax = cols.tile([1, 1], f32, name=f"gm{s}{br}", tag="c0")
                nc.vector.tensor_reduce(out=gmax[:], in_=rmt[:], axis=X,
                                        op=Alu.max)
                ngmax = cols.tile([1, 1], f32, name=f"ngm{s}{br}", tag="c0")
                nc.scalar.mul(ngmax[:], gmax[:], -1.0)
                nm_ps = ps_ty.tile([128, 1], f32, name=f"nmp{s}{br}", tag="ty")
                nc.tensor.matmul(nm_ps[:], ones_r[:], ngmax[:])
                nmcol = cols.tile([128, 1], f32, name=f"nmc{s}{br}", tag="c1")
                psum_copy_to(nmcol[:], nm_ps[:])
                rowsum = cols.tile([128, 1], f32, name=f"rs{s}{br}", tag="c1")
                nc.scalar.activation(out=e_dst[:], in_=sim_ps[:], func=Act.Exp,
                                     bias=nmcol[:], scale=1.0,
                                     accum_out=rowsum[:])
                tot_ps = ps_ty.tile([1, 1], f32, name=f"tot{s}{br}", tag="ty")
                nc.tensor.matmul(tot_ps[:], rowsum[:], ones_c[:])
                invt = keep.tile([1, 1], f32, name=f"it{s}{br}",
                                 tag=f"it{s}{br}")
                nc.vector.reciprocal(invt[:], tot_ps[:])
                ic_ps = ps_ty.tile([128, 1], f32, name=f"icp{s}{br}", tag="ty")
                nc.tensor.matmul(ic_ps[:], ones_r[:], invt[:])
                iccol = keep.tile([128, 1], f32, name=f"icc{s}{br}",
                                  tag=f"icc{s}{br}")
                psum_copy_to(iccol[:], ic_ps[:])
                it1[(s, br)] = invt
                itc[(s, br)] = iccol

            scht = smap("scht", [128, CT * H], bufs=1)  # [h, (t,c_loc)] -> x_ch^T
            scwt = smap("scwt", [128, CT * W], bufs=1)  # [w, (t,c_loc)] -> x_cw^T

            for s in range(S):
                transpose_to(scht[s], xch[s], CT, f"sch{s}")
                transpose_to(scwt[s], xcw[s], CT, f"scw{s}")
                shp = ps_mm.tile([128, 128], f32, name=f"shp{s}", tag="mm")
                nc.tensor.transpose(shp[:], xhwT[s][:], identity[:])
                psum_copy_to(shw[s][:], shp[:])

                # --- branch 12: sim12[w,c] = sum_h Shw[h,w] Sch[c,h]
                sim12 = ps_mm.tile([128, C], f32, name=f"s12_{s}", tag="mm")
                nc.tensor.matmul(sim12[:], shw[s][:], scht[s][:])
                softmax(s, 12, sim12, C, e12[s])
                transpose_to(e12t[s], e12[s], CT, f"e12{s}")
                # y12T[w,h] = sum_c e12t[c,w]^T ... accumulate 2 c tiles
                y12p = ps_mm.tile([128, H], f32, name=f"y12p{s}", tag="mm")
                for t in range(CT):
                    nc.tensor.matmul(
                        y12p[:], e12t[s][:, t * W:(t + 1) * W],
                        xch[s][:, t * H:(t + 1) * H],
                        start=(t == 0), stop=(t == CT - 1))
                psum_copy_to(y12T[s][:], y12p[:])
                # y21[c,h] per c tile
                for t in range(CT):
                    y21p = ps_mm.tile([128, H], f32, name=f"y21p{s}{t}",
                                      tag="mm")
                    nc.tensor.matmul(y21p[:], e12[s][:, t * 128:(t + 1) * 128],
                                     xhwT[s][:])
                    psum_copy_to(y21[s][:, t * H:(t + 1) * H], y21p[:])

                # --- branch 13: sim13[h,c] = sum_w Shw[h,w] Scw[c,w]
                sim13 = ps_mm.tile([128, C], f32, name=f"s13_{s}", tag="mm")
                nc.tensor.matmul(sim13[:], xhwT[s][:], scwt[s][:])
                softmax(s, 13, sim13, C, e13[s])
                transpose_to(e13t[s], e13[s], CT, f"e13{s}")
                y13p = ps_mm.tile([128, W], f32, name=f"y13p{s}", tag="mm")
                for t in range(CT):
                    nc.tensor.matmul(
                        y13p[:], e13t[s][:, t * H:(t + 1) * H],
                        xcw[s][:, t * W:(t + 1) * W],
                        start=(t == 0), stop=(t == CT - 1))
                psum_copy_to(y13[s][:], y13p[:])
                for t in range(CT):
                    y31p = ps_mm.tile([128, W], f32, name=f"y31p{s}{t}",
                                      tag="mm")
                    nc.tensor.matmul(y31p[:], e13[s][:, t * 128:(t + 1) * 128],
                                     shw[s][:])
                    psum_copy_to(y31[s][:, t * W:(t + 1) * W], y31p[:])

                # --- branch 23: sim23[h,w] = sum_c Sch[c,h] Scw[c,w]
                sim23 = ps_mm.tile([128, W], f32, name=f"s23_{s}", tag="mm")
                for t in range(CT):
                    nc.tensor.matmul(
                        sim23[:], xch[s][:, t * H:(t + 1) * H],
                        xcw[s][:, t * W:(t + 1) * W],
                        start=(t == 0), stop=(t == CT - 1))
                softmax(s, 23, sim23, W, e23[s])
                transpose_to(e23t[s], e23[s], 1, f"e23{s}")
                for t in range(CT):
                    y23p = ps_mm.tile([128, H], f32, name=f"y23p{s}{t}",
                                      tag="mm")
                    nc.tensor.matmul(y23p[:], scwt[s][:, t * W:(t + 1) * W],
                                     e23t[s][:])
                    psum_copy_to(y23[s][:, t * H:(t + 1) * H], y23p[:])
                    y32p = ps_mm.tile([128, W], f32, name=f"y32p{s}{t}",
                                      tag="mm")
                    nc.tensor.matmul(y32p[:], scht[s][:, t * H:(t + 1) * H],
                                     e23[s][:])
                    psum_copy_to(y32[s][:, t * W:(t + 1) * W], y32p[:])

                # y12 = transpose(y12T)
                y12pp = ps_mm.tile([128, 128], f32, name=f"y12pp{s}", tag="mm")
                nc.tensor.transpose(y12pp[:], y12T[s][:], identity[:])
                psum_copy_to(y12[s][:], y12pp[:])

            if "C" not in phases:
                return

            # ---------------- phase C: BN partials ----------------
            ysq = gscr.tile([128, 128], f32, name="ysq", tag="ysq", bufs=2)
            for s in range(S):
                it2 = {}
                for br in (12, 13, 23):
                    t2 = keep.tile([128, 1], f32, name=f"it2_{s}{br}",
                                   tag=f"it2_{s}{br}")
                    nc.vector.tensor_tensor(out=t2[:], in0=itc[(s, br)][:],
                                            in1=itc[(s, br)][:], op=Alu.mult)
                    it2[br] = t2
                bnc_maps = [(0, y21[s], 12), (1, y23[s], 23),
                            (2, y31[s], 13), (3, y32[s], 23)]
                for m, ysb, br in bnc_maps:
                    r2 = cols.tile([128, 2], f32, name=f"r{s}{m}", tag="c2")
                    nc.vector.tensor_reduce(
                        out=r2[:], in_=ysb[:].rearrange("p (t h) -> p t h",
                                                        t=CT),
                        axis=X, op=Alu.add)
                    nc.vector.scalar_tensor_tensor(
                        out=bnp[:, m * 2:m * 2 + 2], in0=r2[:],
                        scalar=itc[(s, br)][:], in1=bnp[:, m * 2:m * 2 + 2],
                        op0=Alu.mult, op1=Alu.add)
                    for t in range(CT):
                        col = m * 2 + t
                        blk = ysb[:, t * 128:(t + 1) * 128]
                        sq = cols.tile([128, 1], f32, name=f"sq{s}{m}{t}",
                                       tag="c1")
                        nc.scalar.activation(out=ysq[:], in_=blk,
                                             func=Act.Square, accum_out=sq[:])
                        nc.vector.scalar_tensor_tensor(
                            out=bnp[:, 8 + col:9 + col], in0=sq[:],
                            scalar=it2[br][:], in1=bnp[:, 8 + col:9 + col],
                            op0=Alu.mult, op1=Alu.add)
                # bn1 partials (partition 0, cols 16..19)
                for j, (ymap, br) in enumerate(((y12T[s], 12), (y13[s], 13))):
                    i1 = it1[(s, br)]
                    i2 = cols.tile([1, 1], f32, name=f"i2_{s}{j}", tag="c0")
                    nc.vector.tensor_tensor(out=i2[:], in0=i1[:], in1=i1[:],
                                            op=Alu.mult)
                    rs = cols.tile([128, 1], f32, name=f"rs1_{s}{j}", tag="c1")
                    nc.vector.tensor_reduce(out=rs[:], in_=ymap[:], axis=X,
                                            op=Alu.add)
                    tp = ps_ty.tile([1, 1], f32, name=f"t1_{s}{j}", tag="ty")
                    nc.tensor.matmul(tp[:], rs[:], ones_c[:])
                    nc.vector.scalar_tensor_tensor(
                        out=bnp[0:1, 16 + 2 * j:17 + 2 * j], in0=tp[:],
                        scalar=i1[:], in1=bnp[0:1, 16 + 2 * j:17 + 2 * j],
                        op0=Alu.mult, op1=Alu.add)
                    sqc = cols.tile([128, 1], f32, name=f"sqc{s}{j}", tag="c1")
                    nc.scalar.activation(out=ysq[:], in_=ymap[:],
                                         func=Act.Square, accum_out=sqc[:])
                    tp2 = ps_ty.tile([1, 1], f32, name=f"t2_{s}{j}", tag="ty")
                    nc.tensor.matmul(tp2[:], sqc[:], ones_c[:])
                    nc.vector.scalar_tensor_tensor(
                        out=bnp[0:1, 17 + 2 * j:18 + 2 * j], in0=tp2[:],
                        scalar=i2[:], in1=bnp[0:1, 17 + 2 * j:18 + 2 * j],
                        op0=Alu.mult, op1=Alu.add)

            # ---------------- allreduce ----------------
            nc.sync.dma_start(ccin, bnp[:])
            if n_cores > 1:
                nc.gpsimd.collective_compute(
                    "AllReduce", Alu.add,
                    replica_groups=[list(range(n_cores))],
                    ins=[ccin], outs=[ccout])
            else:
                nc.sync.dma_start(ccout, ccin)
            bnpg = persist.tile([128, NP], f32)
            nc.sync.dma_start(bnpg[:], ccout)

            # ---------------- phase D: BN finalize + gates ----------------
            sm = persist.tile([128, 8], f32, name="mu8")
            nc.vector.tensor_scalar_mul(sm[:], bnpg[:, 0:8], 1.0 / ncn)
            m2 = persist.tile([128, 8], f32, name="m28")
            nc.vector.tensor_tensor(out=m2[:], in0=sm[:], in1=sm[:],
                                    op=Alu.mult)
            var8 = persist.tile([128, 8], f32, name="var8")
            nc.vector.scalar_tensor_tensor(
                out=var8[:], in0=bnpg[:, 8:16], scalar=1.0 / ncn, in1=m2[:],
                op0=Alu.mult, op1=Alu.subtract)
            sd8 = persist.tile([128, 8], f32, name="sd8")
            nc.scalar.activation(out=sd8[:], in_=var8[:], func=Act.Sqrt,
                                 bias=eps_col[:])
            rstd8 = persist.tile([128, 8], f32, name="rstd8")
            nc.vector.reciprocal(rstd8[:], sd8[:])
            scale8 = persist.tile([128, 8], f32, name="scale8")
            nc.vector.tensor_tensor(out=scale8[:], in0=rstd8[:], in1=wc8[:],
                                    op=Alu.mult)
            q8 = persist.tile([128, 8], f32, name="q8")
            nc.vector.tensor_tensor(out=q8[:], in0=sm[:], in1=scale8[:],
                                    op=Alu.mult)
            shift8 = persist.tile([128, 8], f32, name="shift8")
            nc.vector.scalar_tensor_tensor(
                out=shift8[:], in0=q8[:], scalar=-1.0, in1=bc8[:],
                op0=Alu.mult, op1=Alu.add)

            # bn1 scalars on partition 0 (j=0 -> y12, j=1 -> y13)
            sc1 = []
            sh1col = []
            for j in range(2):
                mu1 = cols.tile([1, 1], f32, name=f"mu1_{j}", tag="c0")
                nc.vector.tensor_scalar_mul(mu1[:], bnpg[0:1, 16 + 2 * j:17 + 2 * j],
                                            1.0 / n1)
                m21 = cols.tile([1, 1], f32, name=f"m21_{j}", tag="c0")
                nc.vector.tensor_tensor(out=m21[:], in0=mu1[:], in1=mu1[:],
                                        op=Alu.mult)
                v1 = cols.tile([1, 1], f32, name=f"v1_{j}", tag="c0")
                nc.vector.scalar_tensor_tensor(
                    out=v1[:], in0=bnpg[0:1, 17 + 2 * j:18 + 2 * j],
                    scalar=1.0 / n1, in1=m21[:], op0=Alu.mult,
                    op1=Alu.subtract)
                sd1 = cols.tile([1, 1], f32, name=f"sd1_{j}", tag="c0")
                nc.scalar.activation(out=sd1[:], in_=v1[:], func=Act.Sqrt,
                                     bias=eps_col[0:1, :])
                rst1 = cols.tile([1, 1], f32, name=f"rst1_{j}", tag="c0")
                nc.vector.reciprocal(rst1[:], sd1[:])
                sc = keep.tile([1, 1], f32, name=f"sc1_{j}",
                               tag=f"sc1_{j}")
                nc.vector.tensor_tensor(out=sc[:], in0=rst1[:], in1=bn1w_sb[:],
                                        op=Alu.mult)
                sc1.append(sc)
                q1 = cols.tile([1, 1], f32, name=f"q1_{j}", tag="c0")
                nc.vector.tensor_tensor(out=q1[:], in0=mu1[:], in1=sc[:],
                                        op=Alu.mult)
                sh = cols.tile([1, 1], f32, name=f"sh1_{j}", tag="c0")
                nc.vector.scalar_tensor_tensor(
                    out=sh[:], in0=q1[:], scalar=-1.0, in1=bn1b_sb[:],
                    op0=Alu.mult, op1=Alu.add)
                shp_ = ps_ty.tile([128, 1], f32, name=f"shp1_{j}", tag="ty")
                nc.tensor.matmul(shp_[:], ones_r[:], sh[:])
                shcol = keep.tile([128, 1], f32, name=f"shc1_{j}",
                                  tag=f"shc1_{j}")
                psum_copy_to(shcol[:], shp_[:])
                sh1col.append(shcol)

            bgate = smap("bgate", [128, CT * H], dtype=bf16)
            cgate = smap("cgate", [128, CT * W], dtype=bf16)
            for s in range(S):
                # A gate (bf16), dumped flat to DRAM for the phase-E
                # partition-broadcast reload
                g1 = gscr.tile([128, W], f32, name=f"g12_{s}", tag="ga")
                g2 = gscr.tile([128, W], f32, name=f"g13_{s}", tag="ga")
                for j, (ymap, br, g) in enumerate(
                        ((y12[s], 12, g1), (y13[s], 13, g2))):
                    scs = cols.tile([1, 1], f32, name=f"scs{s}{j}", tag="c0")
                    nc.vector.tensor_tensor(out=scs[:], in0=sc1[j][:],
                                            in1=it1[(s, br)][:], op=Alu.mult)
                    scp = ps_ty.tile([128, 1], f32, name=f"scp{s}{j}",
                                     tag="ty")
                    nc.tensor.matmul(scp[:], ones_r[:], scs[:])
                    sccol = cols.tile([128, 1], f32, name=f"sccol{s}{j}",
                                      tag="c1")
                    psum_copy_to(sccol[:], scp[:])
                    nc.scalar.activation(out=g[:], in_=ymap[:],
                                         func=Act.Sigmoid, bias=sh1col[j][:],
                                         scale=sccol[:])
                nc.vector.tensor_tensor(out=agate[s][:], in0=g1[:], in1=g2[:],
                                        op=Alu.mult)
                nc.scalar.dma_start(
                    adram[s].rearrange("(h w) -> h w", h=H), agate[s][:])
                # B / C gates
                for gate, (ma, bra), (mb, brb), ysa, ysb_ in (
                        (bgate[s], (0, 12), (1, 23), y21[s], y23[s]),
                        (cgate[s], (2, 13), (3, 23), y31[s], y32[s])):
                    ga = gscr.tile([128, CT * 128], f32, name=f"ga{s}{ma}",
                                   tag="gb")
                    gb = gscr.tile([128, CT * 128], f32, name=f"gb{s}{mb}",
                                   tag="gb")
                    for (m, br, ysrc, gdst) in ((ma, bra, ysa, ga),
                                                (mb, brb, ysb_, gb)):
                        for t in range(CT):
                            col = m * 2 + t
                            scc = cols.tile([128, 1], f32,
                                            name=f"scc{s}{m}{t}", tag="c1")
                            nc.vector.tensor_tensor(
                                out=scc[:], in0=scale8[:, col:col + 1],
                                in1=itc[(s, br)][:], op=Alu.mult)
                            nc.scalar.activation(
                                out=gdst[:, t * 128:(t + 1) * 128],
                                in_=ysrc[:, t * 128:(t + 1) * 128],
                                func=Act.Sigmoid,
                                bias=shift8[:, col:col + 1], scale=scc[:])
                    nc.vector.tensor_tensor(out=gate[:], in0=ga[:], in1=gb[:],
                                            op=Alu.mult)

        # ---------------- phase E: apply (partition = channel) ----------------
        if "E" not in phases:
            return
        e_stack = contextlib.ExitStack()
        with e_stack:
            pstr = e_stack.enter_context(tc.tile_pool(name="pstr", bufs=2))
            pme = e_stack.enter_context(tc.tile_pool(name="pme", bufs=3))
            poe = e_stack.enter_context(tc.tile_pool(name="poe", bufs=3))
            paf = e_stack.enter_context(tc.tile_pool(name="paf", bufs=1))

            for s in range(S):
                # replicate A = g12*g13 [h,w] across the 128 c partitions
                # via a stride-0 partition-broadcast DMA load
                afull = paf.tile([128, H, W], bf16, name=f"af{s}", tag="af")
                nc.sync.dma_start(
                    afull[:], adram[s].rearrange("(h w) -> h w", h=H)
                    .unsqueeze(0).broadcast_to([128, H, W]))

                for t in range(CT):
                    bsl_all = bgate[s][:, t * H:(t + 1) * H]
                    csl = cgate[s][:, t * W:(t + 1) * W] \
                        .unsqueeze(1).broadcast_to([128, HC, W])
                    for k in range(NCH):
                        if k < KRES:
                            xsrc = xres[s][t][:, k * HC:(k + 1) * HC, :]
                        else:
                            xt = pstr.tile([128, HC, W], f32,
                                           name=f"xe{s}{t}{k}", tag="xe")
                            nc.sync.dma_start(
                                xt[:], xs[s, t * 128:(t + 1) * 128,
                                          k * HC:(k + 1) * HC, :])
                            xsrc = xt[:]
                        m = pme.tile([128, HC, W], bf16, name=f"me{s}{t}{k}",
                                     tag="me")
                        bsl = bsl_all[:, k * HC:(k + 1) * HC] \
                            .unsqueeze(2).broadcast_to([128, HC, W])
                        nc.gpsimd.tensor_tensor(out=m[:], in0=bsl, in1=csl,
                                                op=Alu.mult)
                        nc.vector.tensor_tensor(
                            out=m[:], in0=m[:],
                            in1=afull[:, k * HC:(k + 1) * HC, :], op=Alu.mult)
                        o = poe.tile([128, HC, W], f32, name=f"oe{s}{t}{k}",
                                     tag="oe")
                        nc.vector.scalar_tensor_tensor(
                            out=o[:], in0=m[:], scalar=1.0, in1=xsrc,
                            op0=Alu.add, op1=Alu.mult)
                        nc.scalar.dma_start(
                            outy[s, t * 128:(t + 1) * 128,
                                 k * HC:(k + 1) * HC, :], o[:])


_NC_CACHE = {}
LAST_RESULT = None


def _get_nc(n_cores: int, sync_start: bool = False, phases: str = "ABCDE"):
    key = (n_cores, sync_start, phases)
    if key not in _NC_CACHE:
        _NC_CACHE[key] = build_bass(n_cores, sync_start, phases)
    return _NC_CACHE[key]


def kernel(**inputs) -> np.ndarray:
    from concourse.bass_utils import run_bass_kernel_spmd

    x = np.ascontiguousarray(inputs["x"], dtype=np.float32)
    bn1_w = np.ascontiguousarray(inputs["bn1_w"], dtype=np.float32)
    bn1_b = np.ascontiguousarray(inputs["bn1_b"], dtype=np.float32)
    bnc_w = np.ascontiguousarray(inputs["bnc_w"], dtype=np.float32)
    bnc_b = np.ascontiguousarray(inputs["bnc_b"], dtype=np.float32)
    B = x.shape[0]
    assert B == NCORES * S, (B, NCORES, S)

    nc = _get_nc(NCORES)
    in_maps = []
    for i in range(NCORES):
        in_maps.append({
            "xs": np.ascontiguousarray(x[i * S:(i + 1) * S]),
            "bn1_w": bn1_w, "bn1_b": bn1_b,
            "bnc_w": bnc_w, "bnc_b": bnc_b,
        })
    res = run_bass_kernel_spmd(nc, in_maps, core_ids=list(range(NCORES)))
    global LAST_RESULT
    LAST_RESULT = res
    out = np.concatenate([res.results[i]["outy"] for i in range(NCORES)],
                         axis=0)
    return out


# revision 21
# speedup vs baseline: 1.5917x; 1.2508x over previous
"""Trainium2 Bass kernel for nn_Cross_Attention (triplet-pool cross-attention gating).

Math (per sample b):
  pools:  Shw[h,w]=max_c x,  Sch[c,h]=max_w x,  Scw[c,w]=max_h x
  3 branches of flat-softmax cross attention between pools -> y12,y13 [h,w],
  y21,y23 [c,h], y31,y32 [c,w]
  training-mode BatchNorm over the *global* batch (cross-core allreduce of
  sum/sumsq), sigmoid gates, and finally
  out = x * (g12*g13)[h,w] * (g21*g23)[c,h] * (g31*g32)[c,w] + x
      = x * (1 + A[h,w]*B[c,h]*Cg[c,w])

Sharding: batch-parallel, 2 samples per core on 8 cores; only the BN batch
stats cross cores (AllReduce of a [128,20] tile).

Apply phase keeps partition=channel so both the x reload and the out store
are contiguous 8KB-per-partition DMAs; A[h,w] is replicated across the 128
channel partitions via PE row-broadcasts (bf16), and the first KRES h-chunks
of x are kept resident in SBUF as bf16 from pass 1 to cut the reload.
"""

import numpy as np

import concourse.bacc as bacc
import concourse.mybir as mybir
import concourse.tile as tile
from concourse import masks

f32 = mybir.dt.float32
bf16 = mybir.dt.bfloat16
Alu = mybir.AluOpType
Act = mybir.ActivationFunctionType
X = mybir.AxisListType.X

NCORES = 8
S = 2          # samples per core
C, H, W = 256, 128, 128
CT = 2         # c tiles of 128
HC = 16        # h rows per chunk
NCH = H // HC  # 8
KRES = 6       # h-chunks per (s,t) kept resident in SBUF as bf16
NP = 20        # bnp columns
EPS = 1e-5


def build_bass(n_cores: int, sync_start: bool = False, phases: str = "ABCDE"):
    """sync_start/phases are for timing probes only: sync_start prepends a
    tiny AllReduce so all cores start main work in lockstep (makes full
    device time visible to the marginal-time harness); phases truncates."""
    nc = bacc.Bacc("TRN2", target_bir_lowering=False, debug=False,
                   num_devices=n_cores)
    nb_tot = n_cores * S
    n1 = float(nb_tot * H * W)   # bn1 count
    ncn = float(nb_tot * H)      # bnc count (per channel)

    xs = nc.dram_tensor("xs", [S, C, H, W], f32, kind="ExternalInput").ap()
    bn1w = nc.dram_tensor("bn1_w", [1], f32, kind="ExternalInput").ap()
    bn1b = nc.dram_tensor("bn1_b", [1], f32, kind="ExternalInput").ap()
    bncw = nc.dram_tensor("bnc_w", [C], f32, kind="ExternalInput").ap()
    bncb = nc.dram_tensor("bnc_b", [C], f32, kind="ExternalInput").ap()
    outy = nc.dram_tensor("outy", [S, C, H, W], bf16,
                          kind="ExternalOutput").ap()

    ccin = nc.dram_tensor("ccin", [128, NP], f32).ap()
    ccout = nc.dram_tensor(
        "ccout", [128, NP], f32,
        addr_space="Shared" if n_cores > 1 else "Local").ap()
    adram = nc.dram_tensor("adram", [S, H * W], bf16).ap()
    sync_bufs = None
    if sync_start:
        sin = nc.dram_tensor("sin", [1, 1], f32).ap()
        sout = nc.dram_tensor(
            "sout", [1, 1], f32,
            addr_space="Shared" if n_cores > 1 else "Local").ap()
        sync_bufs = (sin, sout)

    with tile.TileContext(nc) as tc:
        _emit(nc, tc, n_cores, n1, ncn,
              xs, bn1w, bn1b, bncw, bncb, outy, ccin, ccout, adram,
              sync_bufs, phases)
    nc.compile()
    return nc


def _emit(nc, tc, n_cores, n1, ncn,
          xs, bn1w, bn1b, bncw, bncb, outy, ccin, ccout, adram,
          sync_bufs=None, phases="ABCDE"):
    import contextlib
    stack = contextlib.ExitStack()
    with stack:
        persist = stack.enter_context(tc.tile_pool(name="persist", bufs=1))
        maps = stack.enter_context(tc.tile_pool(name="maps", bufs=2))
        cols = stack.enter_context(tc.tile_pool(name="cols", bufs=4))
        keep = stack.enter_context(tc.tile_pool(name="keep", bufs=1))
        gscr = stack.enter_context(tc.tile_pool(name="gscr", bufs=4))

        # --- timing-only start barrier: a tiny AllReduce whose result is
        # loaded on the sync DMA queue, so every later HWDGE load (FIFO per
        # engine) waits until all cores have started this iteration ---
        if sync_bufs is not None:
            sin, sout = sync_bufs
            st0 = persist.tile([1, 1], f32, name="st0")
            nc.vector.memset(st0[:], 1.0)
            nc.sync.dma_start(sin, st0[:])
            nc.gpsimd.collective_compute(
                "AllReduce", Alu.add,
                replica_groups=[list(range(n_cores))],
                ins=[sin], outs=[sout])
            st1 = persist.tile([1, 1], f32, name="st1")
            nc.sync.dma_start(st1[:], sout)

        # --- setup ---
        identity = persist.tile([128, 128], f32)
        masks.make_identity(nc, identity[:])
        ones_r = persist.tile([1, 128], f32)
        nc.vector.memset(ones_r[:], 1.0)
        ones_c = persist.tile([128, 1], f32)
        nc.vector.memset(ones_c[:], 1.0)
        eps_col = persist.tile([128, 1], f32)
        nc.vector.memset(eps_col[:], EPS)
        wc2 = persist.tile([128, 2], f32)
        nc.sync.dma_start(wc2[:], bncw.rearrange("(t c) -> c t", c=128))
        bc2 = persist.tile([128, 2], f32)
        nc.sync.dma_start(bc2[:], bncb.rearrange("(t c) -> c t", c=128))
        bn1w_sb = persist.tile([1, 1], f32)
        nc.sync.dma_start(bn1w_sb[:], bn1w.unsqueeze(1))
        bn1b_sb = persist.tile([1, 1], f32)
        nc.sync.dma_start(bn1b_sb[:], bn1b.unsqueeze(1))
        wc8 = persist.tile([128, 8], f32)
        bc8 = persist.tile([128, 8], f32)
        for m in range(4):
            nc.vector.tensor_copy(wc8[:, m * 2:m * 2 + 2], wc2[:])
            nc.vector.tensor_copy(bc8[:, m * 2:m * 2 + 2], bc2[:])
        bnp = persist.tile([128, NP], f32)
        nc.vector.memset(bnp[:], 0.0)

        # per-sample persistent maps (bufs=2 -> one slot per sample)
        def smap(name, shape, bufs=None, dtype=f32):
            return [maps.tile(shape, dtype, name=f"{name}{s}", tag=name,
                              bufs=bufs)
                    for s in range(S)]

        xch = smap("xch", [128, CT * H])    # [c_loc, (t,h)]
        xcw = smap("xcw", [128, CT * W])    # [c_loc, (t,w)]
        xhwT = smap("xhwT", [128, H])       # [w, h]
        shw = smap("shw", [128, W])         # [h, w]
        e12 = smap("e12", [128, C], bufs=1)         # [w, c]
        e12t = smap("e12t", [128, CT * W], bufs=1)  # [c_loc, (t,w)]
        e13 = smap("e13", [128, C], bufs=1)         # [h, c]
        e13t = smap("e13t", [128, CT * H], bufs=1)  # [c_loc, (t,h)]
        e23 = smap("e23", [128, W], bufs=1)         # [h, w]
        e23t = smap("e23t", [128, H], bufs=1)       # [w, h]
        y12T = smap("y12T", [128, H], bufs=1)       # [w, h]
        y12 = smap("y12", [128, W])         # [h, w]
        y13 = smap("y13", [128, W])         # [h, w]
        y21 = smap("y21", [128, CT * H])    # [c_loc, (t,h)]
        y23 = smap("y23", [128, CT * H])
        y31 = smap("y31", [128, CT * W])    # [c_loc, (t,w)]
        y32 = smap("y32", [128, CT * W])
        agate = smap("agate", [128, W], dtype=bf16)     # [h, w]
        itc = {}   # invT cols [128,1] per (s, branch)
        it1 = {}   # invT [1,1] per (s, branch)

        # resident bf16 x chunks: [c_loc, k, h_sub, w] per (s, t)
        xres = [[persist.tile([128, KRES * HC, W], bf16,
                              name=f"xres{s}{t}", tag=f"xres{s}{t}")
                 for t in range(CT)] for s in range(S)] if KRES else None

        ps_stack = contextlib.ExitStack()
        with ps_stack:
            px = ps_stack.enter_context(tc.tile_pool(name="px", bufs=3))
            pm = ps_stack.enter_context(tc.tile_pool(name="pm", bufs=3))
            pxcw = ps_stack.enter_context(tc.tile_pool(name="pxcw", bufs=2))
            ps_t = ps_stack.enter_context(
                tc.tile_pool(name="ps_t", bufs=2, space="PSUM"))
            ps_mm = ps_stack.enter_context(
                tc.tile_pool(name="ps_mm", bufs=2, space="PSUM"))
            ps_ty = ps_stack.enter_context(
                tc.tile_pool(name="ps_ty", bufs=2, space="PSUM"))

            # ---------------- pass 1: pooled descriptors ----------------
            for s in range(S):
                xcwp = [pxcw.tile([128, NCH * W], f32, name=f"xcwp{s}{t}",
                                  tag="xcwp") for t in range(CT)]
                for k in range(NCH):
                    xts = []
                    for t in range(CT):
                        xt = px.tile([128, HC, W], f32, name=f"xt{s}{t}{k}",
                                     tag="xt")
                        nc.sync.dma_start(
                            xt[:], xs[s, t * 128:(t + 1) * 128,
                                      k * HC:(k + 1) * HC, :])
                        xts.append(xt)
                        # x_ch partial: max over w
                        nc.vector.tensor_reduce(
                            out=xch[s][:, t * H + k * HC: t * H + (k + 1) * HC],
                            in_=xt[:], axis=X, op=Alu.max)
                        # x_cw partial: max over h-sub
                        nc.vector.tensor_reduce(
                            out=xcwp[t][:, k * W:(k + 1) * W],
                            in_=xt[:].transpose([0, 2, 1]), axis=X, op=Alu.max)
                        # resident bf16 copy for the apply phase
                        if k < KRES:
                            nc.scalar.copy(
                                xres[s][t][:, k * HC:(k + 1) * HC, :], xt[:])
                    # fold the two c tiles for x_hw
                    mch = pm.tile([128, HC, W], f32, name=f"m{s}{k}", tag="m")
                    nc.vector.tensor_tensor(
                        out=mch[:], in0=xts[0][:], in1=xts[1][:], op=Alu.max)
                    for g8 in range(HC // 8):
                        tr = ps_t.tile([128, 8, 128], f32, name=f"tr{s}{k}{g8}",
                                       tag="tr")
                        for j in range(8):
                            nc.tensor.transpose(
                                tr[:, j, :], mch[:, g8 * 8 + j, :], identity[:])
                        nc.vector.tensor_reduce(
                            out=xhwT[s][:, k * HC + g8 * 8: k * HC + g8 * 8 + 8],
                            in_=tr[:], axis=X, op=Alu.max)
                for t in range(CT):
                    nc.vector.tensor_reduce(
                        out=xcw[s][:, t * W:(t + 1) * W],
                        in_=xcwp[t][:].rearrange("p (k w) -> p w k", w=W),
                        axis=X, op=Alu.max)

            if "B" not in phases:
                return

            # ---------------- phase B: attention ----------------
            def psum_copy_to(dst, src_ps):
                nc.scalar.copy(dst, src_ps)

            def transpose_to(dst, src_sb, nblk, name):
                # src [128, nblk*128] -> dst [128, nblk*128] blockwise T
                for t in range(nblk):
                    tp = ps_mm.tile([128, 128], f32, name=f"tp{name}{t}",
                                    tag="mm")
                    nc.tensor.transpose(
                        tp[:], src_sb[:, t * 128:(t + 1) * 128], identity[:])
                    psum_copy_to(dst[:, t * 128:(t + 1) * 128], tp[:])

            def softmax(s, br, sim_ps, ncol, e_dst):
                rowmax = cols.tile([128, 1], f32, name=f"rm{s}{br}", tag="c1")
                nc.vector.tensor_reduce(out=rowmax[:], in_=sim_ps[:], axis=X,
                                        op=Alu.max)
                rmt = ps_ty.tile([1, 128], f32, name=f"rmt{s}{br}", tag="ty")
                nc.tensor.transpose(rmt[:], rowmax[:], identity[:])
                gmax = cols.tile([1, 1], f32, name=f"gm{s}{br}", tag="c0")
                nc.vector.tensor_reduce(out=gmax[:], in_=rmt[:], axis=X,
                                        op=Alu.max)
                ngmax = cols.tile([1, 1], f32, name=f"ngm{s}{br}", tag="c0")
                nc.scalar.mul(ngmax[:], gmax[:], -1.0)
                nm_ps = ps_ty.tile([128, 1], f32, name=f"nmp{s}{br}", tag="ty")
                nc.tensor.matmul(nm_ps[:], ones_r[:], ngmax[:])
                nmcol = cols.tile([128, 1], f32, name=f"nmc{s}{br}", tag="c1")
                psum_copy_to(nmcol[:], nm_ps[:])
                rowsum = cols.tile([128, 1], f32, name=f"rs{s}{br}", tag="c1")
                nc.scalar.activation(out=e_dst[:], in_=sim_ps[:], func=Act.Exp,
                                     bias=nmcol[:], scale=1.0,
                                     accum_out=rowsum[:])
                tot_ps = ps_ty.tile([1, 1], f32, name=f"tot{s}{br}", tag="ty")
                nc.tensor.matmul(tot_ps[:], rowsum[:], ones_c[:])
                invt = keep.tile([1, 1], f32, name=f"it{s}{br}",
                                 tag=f"it{s}{br}")
                nc.vector.reciprocal(invt[:], tot_ps[:])
                ic_ps = ps_ty.tile([128, 1], f32, name=f"icp{s}{br}", tag="ty")
                nc.tensor.matmul(ic_ps[:], ones_r[:], invt[:])
                iccol = keep.tile([128, 1], f32, name=f"icc{s}{br}",
                                  tag=f"icc{s}{br}")
                psum_copy_to(iccol[:], ic_ps[:])
                it1[(s, br)] = invt
                itc[(s, br)] = iccol

            scht = smap("scht", [128, CT * H], bufs=1)  # [h, (t,c_loc)] -> x_ch^T
            scwt = smap("scwt", [128, CT * W], bufs=1)  # [w, (t,c_loc)] -> x_cw^T

            for s in range(S):
                transpose_to(scht[s], xch[s], CT, f"sch{s}")
                transpose_to(scwt[s], xcw[s], CT, f"scw{s}")
                shp = ps_mm.tile([128, 128], f32, name=f"shp{s}", tag="mm")
                nc.tensor.transpose(shp[:], xhwT[s][:], identity[:])
                psum_copy_to(shw[s][:], shp[:])

                # --- branch 12: sim12[w,c] = sum_h Shw[h,w] Sch[c,h]
                sim12 = ps_mm.tile([128, C], f32, name=f"s12_{s}", tag="mm")
                nc.tensor.matmul(sim12[:], shw[s][:], scht[s][:])
                softmax(s, 12, sim12, C, e12[s])
                transpose_to(e12t[s], e12[s], CT, f"e12{s}")
                # y12T[w,h] = sum_c e12t[c,w]^T ... accumulate 2 c tiles
                y12p = ps_mm.tile([128, H], f32, name=f"y12p{s}", tag="mm")
                for t in range(CT):
                    nc.tensor.matmul(
                        y12p[:], e12t[s][:, t * W:(t + 1) * W],
                        xch[s][:, t * H:(t + 1) * H],
                        start=(t == 0), stop=(t == CT - 1))
                psum_copy_to(y12T[s][:], y12p[:])
                # y21[c,h] per c tile
                for t in range(CT):
                    y21p = ps_mm.tile([128, H], f32, name=f"y21p{s}{t}",
                                      tag="mm")
                    nc.tensor.matmul(y21p[:], e12[s][:, t * 128:(t + 1) * 128],
                                     xhwT[s][:])
                    psum_copy_to(y21[s][:, t * H:(t + 1) * H], y21p[:])

                # --- branch 13: sim13[h,c] = sum_w Shw[h,w] Scw[c,w]
                sim13 = ps_mm.tile([128, C], f32, name=f"s13_{s}", tag="mm")
                nc.tensor.matmul(sim13[:], xhwT[s][:], scwt[s][:])
                softmax(s, 13, sim13, C, e13[s])
                transpose_to(e13t[s], e13[s], CT, f"e13{s}")
                y13p = ps_mm.tile([128, W], f32, name=f"y13p{s}", tag="mm")
                for t in range(CT):
                    nc.tensor.matmul(
                        y13p[:], e13t[s][:, t * H:(t + 1) * H],
                        xcw[s][:, t * W:(t + 1) * W],
                        start=(t == 0), stop=(t == CT - 1))
                psum_copy_to(y13[s][:], y13p[:])
                for t in range(CT):
                    y31p = ps_mm.tile([128, W], f32, name=f"y31p{s}{t}",
                                      tag="mm")
                    nc.tensor.matmul(y31p[:], e13[s][:, t * 128:(t + 1) * 128],
                                     shw[s][:])
                    psum_copy_to(y31[s][:, t * W:(t + 1) * W], y31p[:])

                # --- branch 23: sim23[h,w] = sum_c Sch[c,h] Scw[c,w]
                sim23 = ps_mm.tile([128, W], f32, name=f"s23_{s}", tag="mm")
                for t in range(CT):
                    nc.tensor.matmul(
                        sim23[:], xch[s][:, t * H:(t + 1) * H],
                        xcw[s][:, t * W:(t + 1) * W],
                        start=(t == 0), stop=(t == CT - 1))
                softmax(s, 23, sim23, W, e23[s])
                transpose_to(e23t[s], e23[s], 1, f"e23{s}")
                for t in range(CT):
                    y23p = ps_mm.tile([128, H], f32, name=f"y23p{s}{t}",
                                      tag="mm")
                    nc.tensor.matmul(y23p[:], scwt[s][:, t * W:(t + 1) * W],
                                     e23t[s][:])
                    psum_copy_to(y23[s][:, t * H:(t + 1) * H], y23p[:])
                    y32p = ps_mm.tile([128, W], f32, name=f"y32p{s}{t}",
                                      tag="mm")
                    nc.tensor.matmul(y32p[:], scht[s][:, t * H:(t + 1) * H],
                                     e23[s][:])
                    psum_copy_to(y32[s][:, t * W:(t + 1) * W], y32p[:])

                # y12 = transpose(y12T)
                y12pp = ps_mm.tile([128, 128], f32, name=f"y12pp{s}", tag="mm")
                nc.tensor.transpose(y12pp[:], y12T[s][:], identity[:])
                psum_copy_to(y12[s][:], y12pp[:])

            if "C" not in phases:
                return

            # ---------------- phase C: BN partials ----------------
            ysq = gscr.tile([128, 128], f32, name="ysq", tag="ysq", bufs=2)
            for s in range(S):
                it2 = {}
                for br in (12, 13, 23):
                    t2 = keep.tile([128, 1], f32, name=f"it2_{s}{br}",
                                   tag=f"it2_{s}{br}")
                    nc.vector.tensor_tensor(out=t2[:], in0=itc[(s, br)][:],
                                            in1=itc[(s, br)][:], op=Alu.mult)
                    it2[br] = t2
                bnc_maps = [(0, y21[s], 12), (1, y23[s], 23),
                            (2, y31[s], 13), (3, y32[s], 23)]
                for m, ysb, br in bnc_maps:
                    r2 = cols.tile([128, 2], f32, name=f"r{s}{m}", tag="c2")
                    nc.vector.tensor_reduce(
                        out=r2[:], in_=ysb[:].rearrange("p (t h) -> p t h",
                                                        t=CT),
                        axis=X, op=Alu.add)
                    nc.vector.scalar_tensor_tensor(
                        out=bnp[:, m * 2:m * 2 + 2], in0=r2[:],
                        scalar=itc[(s, br)][:], in1=bnp[:, m * 2:m * 2 + 2],
                        op0=Alu.mult, op1=Alu.add)
                    for t in range(CT):
                        col = m * 2 + t
                        blk = ysb[:, t * 128:(t + 1) * 128]
                        sq = cols.tile([128, 1], f32, name=f"sq{s}{m}{t}",
                                       tag="c1")
                        nc.scalar.activation(out=ysq[:], in_=blk,
                                             func=Act.Square, accum_out=sq[:])
                        nc.vector.scalar_tensor_tensor(
                            out=bnp[:, 8 + col:9 + col], in0=sq[:],
                            scalar=it2[br][:], in1=bnp[:, 8 + col:9 + col],
                            op0=Alu.mult, op1=Alu.add)
                # bn1 partials (partition 0, cols 16..19)
                for j, (ymap, br) in enumerate(((y12T[s], 12), (y13[s], 13))):
                    i1 = it1[(s, br)]
                    i2 = cols.tile([1, 1], f32, name=f"i2_{s}{j}", tag="c0")
                    nc.vector.tensor_tensor(out=i2[:], in0=i1[:], in1=i1[:],
                                            op=Alu.mult)
                    rs = cols.tile([128, 1], f32, name=f"rs1_{s}{j}", tag="c1")
                    nc.vector.tensor_reduce(out=rs[:], in_=ymap[:], axis=X,
                                            op=Alu.add)
                    tp = ps_ty.tile([1, 1], f32, name=f"t1_{s}{j}", tag="ty")
                    nc.tensor.matmul(tp[:], rs[:], ones_c[:])
                    nc.vector.scalar_tensor_tensor(
                        out=bnp[0:1, 16 + 2 * j:17 + 2 * j], in0=tp[:],
                        scalar=i1[:], in1=bnp[0:1, 16 + 2 * j:17 + 2 * j],
                        op0=Alu.mult, op1=Alu.add)
                    sqc = cols.tile([128, 1], f32, name=f"sqc{s}{j}", tag="c1")
                    nc.scalar.activation(out=ysq[:], in_=ymap[:],
                                         func=Act.Square, accum_out=sqc[:])
                    tp2 = ps_ty.tile([1, 1], f32, name=f"t2_{s}{j}", tag="ty")
                    nc.tensor.matmul(tp2[:], sqc[:], ones_c[:])
                    nc.vector.scalar_tensor_tensor(
                        out=bnp[0:1, 17 + 2 * j:18 + 2 * j], in0=tp2[:],
                        scalar=i2[:], in1=bnp[0:1, 17 + 2 * j:18 + 2 * j],
                        op0=Alu.mult, op1=Alu.add)

            # ---------------- allreduce ----------------
            nc.sync.dma_start(ccin, bnp[:])
            if n_cores > 1:
                nc.gpsimd.collective_compute(
                    "AllReduce", Alu.add,
                    replica_groups=[list(range(n_cores))],
                    ins=[ccin], outs=[ccout])
            else:
                nc.sync.dma_start(ccout, ccin)
            bnpg = persist.tile([128, NP], f32)
            nc.sync.dma_start(bnpg[:], ccout)

            # ---------------- phase D: BN finalize + gates ----------------
            sm = persist.tile([128, 8], f32, name="mu8")
            nc.vector.tensor_scalar_mul(sm[:], bnpg[:, 0:8], 1.0 / ncn)
            m2 = persist.tile([128, 8], f32, name="m28")
            nc.vector.tensor_tensor(out=m2[:], in0=sm[:], in1=sm[:],
                                    op=Alu.mult)
            var8 = persist.tile([128, 8], f32, name="var8")
            nc.vector.scalar_tensor_tensor(
                out=var8[:], in0=bnpg[:, 8:16], scalar=1.0 / ncn, in1=m2[:],
                op0=Alu.mult, op1=Alu.subtract)
            sd8 = persist.tile([128, 8], f32, name="sd8")
            nc.scalar.activation(out=sd8[:], in_=var8[:], func=Act.Sqrt,
                                 bias=eps_col[:])
            rstd8 = persist.tile([128, 8], f32, name="rstd8")
            nc.vector.reciprocal(rstd8[:], sd8[:])
            scale8 = persist.tile([128, 8], f32, name="scale8")
            nc.vector.tensor_tensor(out=scale8[:], in0=rstd8[:], in1=wc8[:],
                                    op=Alu.mult)
            q8 = persist.tile([128, 8], f32, name="q8")
            nc.vector.tensor_tensor(out=q8[:], in0=sm[:], in1=scale8[:],
                                    op=Alu.mult)
            shift8 = persist.tile([128, 8], f32, name="shift8")
            nc.vector.scalar_tensor_tensor(
                out=shift8[:], in0=q8[:], scalar=-1.0, in1=bc8[:],
                op0=Alu.mult, op1=Alu.add)

            # bn1 scalars on partition 0 (j=0 -> y12, j=1 -> y13)
            sc1 = []
            sh1col = []
            for j in range(2):
                mu1 = cols.tile([1, 1], f32, name=f"mu1_{j}", tag="c0")
                nc.vector.tensor_scalar_mul(mu1[:], bnpg[0:1, 16 + 2 * j:17 + 2 * j],
                                            1.0 / n1)
                m21 = cols.tile([1, 1], f32, name=f"m21_{j}", tag="c0")
                nc.vector.tensor_tensor(out=m21[:], in0=mu1[:], in1=mu1[:],
                                        op=Alu.mult)
                v1 = cols.tile([1, 1], f32, name=f"v1_{j}", tag="c0")
                nc.vector.scalar_tensor_tensor(
                    out=v1[:], in0=bnpg[0:1, 17 + 2 * j:18 + 2 * j],
                    scalar=1.0 / n1, in1=m21[:], op0=Alu.mult,
                    op1=Alu.subtract)
                sd1 = cols.tile([1, 1], f32, name=f"sd1_{j}", tag="c0")
                nc.scalar.activation(out=sd1[:], in_=v1[:], func=Act.Sqrt,
                                     bias=eps_col[0:1, :])
                rst1 = cols.tile([1, 1], f32, name=f"rst1_{j}", tag="c0")
                nc.vector.reciprocal(rst1[:], sd1[:])
                sc = keep.tile([1, 1], f32, name=f"sc1_{j}",
                               tag=f"sc1_{j}")
                nc.vector.tensor_tensor(out=sc[:], in0=rst1[:], in1=bn1w_sb[:],
                                        op=Alu.mult)
                sc1.append(sc)
                q1 = cols.tile([1, 1], f32, name=f"q1_{j}", tag="c0")
                nc.vector.tensor_tensor(out=q1[:], in0=mu1[:], in1=sc[:],
                                        op=Alu.mult)
                sh = cols.tile([1, 1], f32, name=f"sh1_{j}", tag="c0")
                nc.vector.scalar_tensor_tensor(
                    out=sh[:], in0=q1[:], scalar=-1.0, in1=bn1b_sb[:],
                    op0=Alu.mult, op1=Alu.add)
                shp_ = ps_ty.tile([128, 1], f32, name=f"shp1_{j}", tag="ty")
                nc.tensor.matmul(shp_[:], ones_r[:], sh[:])
                shcol = keep.tile([128, 1], f32, name=f"shc1_{j}",
                                  tag=f"shc1_{j}")
                psum_copy_to(shcol[:], shp_[:])
                sh1col.append(shcol)

            bgate = smap("bgate", [128, CT * H], dtype=bf16)
            cgate = smap("cgate", [128, CT * W], dtype=bf16)
            for s in range(S):
                # A gate (bf16), dumped flat to DRAM for the phase-E
                # partition-broadcast reload
                g1 = gscr.tile([128, W], f32, name=f"g12_{s}", tag="ga")
                g2 = gscr.tile([128, W], f32, name=f"g13_{s}", tag="ga")
                for j, (ymap, br, g) in enumerate(
                        ((y12[s], 12, g1), (y13[s], 13, g2))):
                    scs = cols.tile([1, 1], f32, name=f"scs{s}{j}", tag="c0")
                    nc.vector.tensor_tensor(out=scs[:], in0=sc1[j][:],
                                            in1=it1[(s, br)][:], op=Alu.mult)
                    scp = ps_ty.tile([128, 1], f32, name=f"scp{s}{j}",
                                     tag="ty")
                    nc.tensor.matmul(scp[:], ones_r[:], scs[:])
                    sccol = cols.tile([128, 1], f32, name=f"sccol{s}{j}",
                                      tag="c1")
                    psum_copy_to(sccol[:], scp[:])
                    nc.scalar.activation(out=g[:], in_=ymap[:],
                                         func=Act.Sigmoid, bias=sh1col[j][:],
                                         scale=sccol[:])
                nc.vector.tensor_tensor(out=agate[s][:], in0=g1[:], in1=g2[:],
                                        op=Alu.mult)
                nc.scalar.dma_start(
                    adram[s].rearrange("(h w) -> h w", h=H), agate[s][:])
                # B / C gates
                for gate, (ma, bra), (mb, brb), ysa, ysb_ in (
                        (bgate[s], (0, 12), (1, 23), y21[s], y23[s]),
                        (cgate[s], (2, 13), (3, 23), y31[s], y32[s])):
                    ga = gscr.tile([128, CT * 128], f32, name=f"ga{s}{ma}",
                                   tag="gb")
                    gb = gscr.tile([128, CT * 128], f32, name=f"gb{s}{mb}",
                                   tag="gb")
                    for (m, br, ysrc, gdst) in ((ma, bra, ysa, ga),
                                                (mb, brb, ysb_, gb)):
                        for t in range(CT):
                            col = m * 2 + t
                            scc = cols.tile([128, 1], f32,
                                            name=f"scc{s}{m}{t}", tag="c1")
                            nc.vector.tensor_tensor(
                                out=scc[:], in0=scale8[:, col:col + 1],
                                in1=itc[(s, br)][:], op=Alu.mult)
                            nc.scalar.activation(
                                out=gdst[:, t * 128:(t + 1) * 128],
                                in_=ysrc[:, t * 128:(t + 1) * 128],
                                func=Act.Sigmoid,
                                bias=shift8[:, col:col + 1], scale=scc[:])
                    nc.vector.tensor_tensor(out=gate[:], in0=ga[:], in1=gb[:],
                                            op=Alu.mult)

        # ---------------- phase E: apply (partition = channel) ----------------
        if "E" not in phases:
            return
        e_stack = contextlib.ExitStack()
        with e_stack:
            pstr = e_stack.enter_context(tc.tile_pool(name="pstr", bufs=2))
            pme = e_stack.enter_context(tc.tile_pool(name="pme", bufs=3))
            poe = e_stack.enter_context(tc.tile_pool(name="poe", bufs=3))
            paf = e_stack.enter_context(tc.tile_pool(name="paf", bufs=1))

            for s in range(S):
                # replicate A = g12*g13 [h,w] across the 128 c partitions
                # via a stride-0 partition-broadcast DMA load
                afull = paf.tile([128, H, W], bf16, name=f"af{s}", tag="af")
                nc.sync.dma_start(
                    afull[:], adram[s].rearrange("(h w) -> h w", h=H)
                    .unsqueeze(0).broadcast_to([128, H, W]))

                for t in range(CT):
                    bsl_all = bgate[s][:, t * H:(t + 1) * H]
                    csl = cgate[s][:, t * W:(t + 1) * W] \
                        .unsqueeze(1).broadcast_to([128, HC, W])
                    for k in range(NCH):
                        if k < KRES:
                            xsrc = xres[s][t][:, k * HC:(k + 1) * HC, :]
                        else:
                            xt = pstr.tile([128, HC, W], f32,
                                           name=f"xe{s}{t}{k}", tag="xe")
                            nc.sync.dma_start(
                                xt[:], xs[s, t * 128:(t + 1) * 128,
                                          k * HC:(k + 1) * HC, :])
                            xsrc = xt[:]
                        m = pme.tile([128, HC, W], bf16, name=f"me{s}{t}{k}",
                                     tag="me")
                        bsl = bsl_all[:, k * HC:(k + 1) * HC] \
                            .unsqueeze(2).broadcast_to([128, HC, W])
                        nc.vector.tensor_tensor(out=m[:], in0=bsl, in1=csl,
                                                op=Alu.mult)
                        nc.vector.tensor_tensor(
                            out=m[:], in0=m[:],
                            in1=afull[:, k * HC:(k + 1) * HC, :], op=Alu.mult)
                        o = poe.tile([128, HC, W], bf16, name=f"oe{s}{t}{k}",
                                     tag="oe")
                        nc.vector.scalar_tensor_tensor(
                            out=o[:], in0=m[:], scalar=1.0, in1=xsrc,
                            op0=Alu.add, op1=Alu.mult)
                        nc.scalar.dma_start(
                            outy[s, t * 128:(t + 1) * 128,
                                 k * HC:(k + 1) * HC, :], o[:])


_NC_CACHE = {}
LAST_RESULT = None


def _get_nc(n_cores: int, sync_start: bool = False, phases: str = "ABCDE"):
    key = (n_cores, sync_start, phases)
    if key not in _NC_CACHE:
        _NC_CACHE[key] = build_bass(n_cores, sync_start, phases)
    return _NC_CACHE[key]


def kernel(**inputs) -> np.ndarray:
    from concourse.bass_utils import run_bass_kernel_spmd

    x = np.ascontiguousarray(inputs["x"], dtype=np.float32)
    bn1_w = np.ascontiguousarray(inputs["bn1_w"], dtype=np.float32)
    bn1_b = np.ascontiguousarray(inputs["bn1_b"], dtype=np.float32)
    bnc_w = np.ascontiguousarray(inputs["bnc_w"], dtype=np.float32)
    bnc_b = np.ascontiguousarray(inputs["bnc_b"], dtype=np.float32)
    B = x.shape[0]
    assert B == NCORES * S, (B, NCORES, S)

    nc = _get_nc(NCORES)
    in_maps = []
    for i in range(NCORES):
        in_maps.append({
            "xs": np.ascontiguousarray(x[i * S:(i + 1) * S]),
            "bn1_w": bn1_w, "bn1_b": bn1_b,
            "bnc_w": bnc_w, "bnc_b": bnc_b,
        })
    res = run_bass_kernel_spmd(nc, in_maps, core_ids=list(range(NCORES)))
    global LAST_RESULT
    LAST_RESULT = res
    out = np.concatenate(
        [np.asarray(res.results[i]["outy"]).astype(np.float32)
         for i in range(NCORES)], axis=0)
    return out


# revision 44
# speedup vs baseline: 2.1389x; 1.3437x over previous
"""Trainium2 Bass kernel for nn_Cross_Attention (triplet-pool cross-attention gating).

Math (per sample b):
  pools:  Shw[h,w]=max_c x,  Sch[c,h]=max_w x,  Scw[c,w]=max_h x
  3 branches of flat-softmax cross attention between pools -> y12,y13 [h,w],
  y21,y23 [c,h], y31,y32 [c,w]
  training-mode BatchNorm over the *global* batch (cross-core allreduce of
  sum/sumsq), sigmoid gates, and finally
  out = x * (g12*g13)[h,w] * (g21*g23)[c,h] * (g31*g32)[c,w] + x
      = x * (1 + A[h,w]*B[c,h]*Cg[c,w])

Sharding: batch-parallel, 2 samples per core on 8 cores; only the BN batch
stats cross cores (AllReduce of a [128,20] tile).

Apply phase keeps partition=channel so both the x reload and the out store
are contiguous 8KB-per-partition DMAs; A[h,w] is replicated across the 128
channel partitions via PE row-broadcasts (bf16), and the first KRES h-chunks
of x are kept resident in SBUF as bf16 from pass 1 to cut the reload.
"""

import numpy as np

import concourse.bacc as bacc
import concourse.mybir as mybir
import concourse.tile as tile
from concourse import masks

f32 = mybir.dt.float32
bf16 = mybir.dt.bfloat16
Alu = mybir.AluOpType
Act = mybir.ActivationFunctionType
X = mybir.AxisListType.X

NCORES = 8
S = 2          # samples per core
C, H, W = 256, 128, 128
CT = 2         # c tiles of 128
HC = 16        # h rows per chunk
NCH = H // HC  # 8
KRES = 8       # h-chunks per (s,t) kept resident in SBUF as bf16 (all)
NP = 20        # bnp columns
EPS = 1e-5


def build_bass(n_cores: int, sync_start: bool = False, phases: str = "ABCDE"):
    """sync_start/phases are for timing probes only: sync_start prepends a
    tiny AllReduce so all cores start main work in lockstep (makes full
    device time visible to the marginal-time harness); phases truncates."""
    nc = bacc.Bacc("TRN2", target_bir_lowering=False, debug=False,
                   num_devices=n_cores)
    nb_tot = n_cores * S
    n1 = float(nb_tot * H * W)   # bn1 count
    ncn = float(nb_tot * H)      # bnc count (per channel)

    xs = nc.dram_tensor("xs", [S, C, H, W], f32, kind="ExternalInput").ap()
    bn1w = nc.dram_tensor("bn1_w", [1], f32, kind="ExternalInput").ap()
    bn1b = nc.dram_tensor("bn1_b", [1], f32, kind="ExternalInput").ap()
    bncw = nc.dram_tensor("bnc_w", [C], f32, kind="ExternalInput").ap()
    bncb = nc.dram_tensor("bnc_b", [C], f32, kind="ExternalInput").ap()
    outy = nc.dram_tensor("outy", [S, C, H, W], bf16,
                          kind="ExternalOutput").ap()

    ccin = nc.dram_tensor("ccin", [128, NP], f32).ap()
    ccout = nc.dram_tensor(
        "ccout", [128, NP], f32,
        addr_space="Shared" if n_cores > 1 else "Local").ap()
    adram = nc.dram_tensor("adram", [S, H * W], bf16).ap()
    sync_bufs = None
    if sync_start:
        sin = nc.dram_tensor("sin", [1, 1], f32).ap()
        sout = nc.dram_tensor(
            "sout", [1, 1], f32,
            addr_space="Shared" if n_cores > 1 else "Local").ap()
        sync_bufs = (sin, sout)

    with tile.TileContext(nc) as tc:
        _emit(nc, tc, n_cores, n1, ncn,
              xs, bn1w, bn1b, bncw, bncb, outy, ccin, ccout, adram,
              sync_bufs, phases)
    nc.compile()
    return nc


def _emit(nc, tc, n_cores, n1, ncn,
          xs, bn1w, bn1b, bncw, bncb, outy, ccin, ccout, adram,
          sync_bufs=None, phases="ABCDE"):
    import contextlib
    stack = contextlib.ExitStack()
    with stack:
        persist = stack.enter_context(tc.tile_pool(name="persist", bufs=1))
        maps = stack.enter_context(tc.tile_pool(name="maps", bufs=2))
        cols = stack.enter_context(tc.tile_pool(name="cols", bufs=4))
        keep = stack.enter_context(tc.tile_pool(name="keep", bufs=1))
        gscr = stack.enter_context(tc.tile_pool(name="gscr", bufs=4))

        # --- timing-only start barrier: a tiny AllReduce whose result is
        # loaded on the sync DMA queue, so every later HWDGE load (FIFO per
        # engine) waits until all cores have started this iteration ---
        if sync_bufs is not None:
            sin, sout = sync_bufs
            st0 = cols.tile([1, 1], f32, name="st0", tag="c0")
            nc.vector.memset(st0[:], 1.0)
            nc.sync.dma_start(sin, st0[:])
            nc.gpsimd.collective_compute(
                "AllReduce", Alu.add,
                replica_groups=[list(range(n_cores))],
                ins=[sin], outs=[sout])
            st1 = cols.tile([1, 1], f32, name="st1", tag="c0")
            nc.sync.dma_start(st1[:], sout)
        else:
            st1 = None

        # --- setup ---
        identity = persist.tile([128, 128], f32)
        masks.make_identity(nc, identity[:])
        ones_r = persist.tile([1, 128], f32)
        nc.vector.memset(ones_r[:], 1.0)
        ones_c = persist.tile([128, 1], f32)
        nc.vector.memset(ones_c[:], 1.0)
        eps_col = persist.tile([128, 1], f32)
        nc.vector.memset(eps_col[:], EPS)
        wc2 = persist.tile([128, 2], f32)
        nc.sync.dma_start(wc2[:], bncw.rearrange("(t c) -> c t", c=128))
        bc2 = persist.tile([128, 2], f32)
        nc.sync.dma_start(bc2[:], bncb.rearrange("(t c) -> c t", c=128))
        bn1w_sb = persist.tile([1, 1], f32)
        nc.sync.dma_start(bn1w_sb[:], bn1w.unsqueeze(1))
        bn1b_sb = persist.tile([1, 1], f32)
        nc.sync.dma_start(bn1b_sb[:], bn1b.unsqueeze(1))
        wc8 = persist.tile([128, 8], f32)
        bc8 = persist.tile([128, 8], f32)
        for m in range(4):
            nc.vector.tensor_copy(wc8[:, m * 2:m * 2 + 2], wc2[:])
            nc.vector.tensor_copy(bc8[:, m * 2:m * 2 + 2], bc2[:])
        bnp = persist.tile([128, NP], f32)
        nc.vector.memset(bnp[:], 0.0)

        # per-sample persistent maps (bufs=2 -> one slot per sample)
        def smap(name, shape, bufs=None, dtype=f32):
            return [maps.tile(shape, dtype, name=f"{name}{s}", tag=name,
                              bufs=bufs)
                    for s in range(S)]

        xch = smap("xch", [128, CT * H])    # [c_loc, (t,h)]
        xcw = smap("xcw", [128, CT * W])    # [c_loc, (t,w)]
        xhwT = smap("xhwT", [128, H])       # [w, h]
        shw = smap("shw", [128, W])         # [h, w]
        y12 = smap("y12", [128, W])         # [h, w]
        y13 = smap("y13", [128, W])         # [h, w]
        # the four bnc y-maps live in one [128, (m,t)*128] tile per sample,
        # cols (m, t): m0=y21(br12) m1=y23(br23) m2=y31(br13) m3=y32(br23),
        # so the whole B/C gate batch is one sigmoid activation later
        Y8 = smap("Y8", [128, 8 * 128])

        def ymt(s, m, t):
            return Y8[s][:, (2 * m + t) * 128:(2 * m + t + 1) * 128]

        def ym(s, m):
            return Y8[s][:, 2 * m * 128:(2 * m + 2) * 128]
        agate = smap("agate", [128, W], dtype=bf16)     # [h, w]
        itc = {}   # invT cols [128,1] per (s, branch)
        it1 = {}   # invT [1,1] per (s, branch)

        # resident bf16 x chunks: [c_loc, k, h_sub, w] per (s, t)
        xres = [[persist.tile([128, KRES * HC, W], bf16,
                              name=f"xres{s}{t}", tag=f"xres{s}{t}")
                 for t in range(CT)] for s in range(S)] if KRES else None

        ps_stack = contextlib.ExitStack()
        with ps_stack:
            px = ps_stack.enter_context(tc.tile_pool(name="px", bufs=3))
            pxcw = ps_stack.enter_context(tc.tile_pool(name="pxcw", bufs=2))
            pbm = ps_stack.enter_context(tc.tile_pool(name="pbm", bufs=1))

            def bmap(name, shape):
                return [pbm.tile(shape, f32, name=f"{name}{s}", tag=name)
                        for s in range(S)]

            e12 = bmap("e12", [128, C])         # [w, c]
            e12t = bmap("e12t", [128, CT * W])  # [c_loc, (t,w)]
            e13 = bmap("e13", [128, C])         # [h, c]
            e13t = bmap("e13t", [128, CT * H])  # [c_loc, (t,h)]
            e23 = bmap("e23", [128, W])         # [h, w]
            e23t = bmap("e23t", [128, H])       # [w, h]
            y12T = bmap("y12T", [128, H])       # [w, h]
            ps_t = ps_stack.enter_context(
                tc.tile_pool(name="ps_t", bufs=2, space="PSUM"))
            ps_mm = ps_stack.enter_context(
                tc.tile_pool(name="ps_mm", bufs=2, space="PSUM"))
            ps_ty = ps_stack.enter_context(
                tc.tile_pool(name="ps_ty", bufs=2, space="PSUM"))

            # ---------------- pass 1: pooled descriptors ----------------
            for s in range(S):
                xcwp = [pxcw.tile([128, NCH * W], f32, name=f"xcwp{s}{t}",
                                  tag="xcwp") for t in range(CT)]
                for k in range(NCH):
                    xts = []
                    for t in range(CT):
                        xt = px.tile([128, HC, W], f32, name=f"xt{s}{t}{k}",
                                     tag="xt")
                        if st1 is not None:
                            # timing probe: force every load to wait for the
                            # start barrier via a WAW dep on the tile
                            nc.vector.tensor_copy(xt[0:1, 0, 0:1], st1[:])
                        nc.sync.dma_start(
                            xt[:], xs[s, t * 128:(t + 1) * 128,
                                      k * HC:(k + 1) * HC, :])
                        xts.append(xt)
                        # x_ch partial: max over w
                        nc.vector.tensor_reduce(
                            out=xch[s][:, t * H + k * HC: t * H + (k + 1) * HC],
                            in_=xt[:], axis=X, op=Alu.max)
                        # x_cw partial: max over h-sub
                        nc.vector.tensor_reduce(
                            out=xcwp[t][:, k * W:(k + 1) * W],
                            in_=xt[:].transpose([0, 2, 1]), axis=X, op=Alu.max)
                        # resident bf16 copy for the apply phase, split
                        # across the scalar and gpsimd engines
                        if k < KRES:
                            dst = xres[s][t][:, k * HC:(k + 1) * HC, :]
                            if t == 0:
                                nc.scalar.copy(dst, xt[:])
                            else:
                                nc.gpsimd.tensor_copy(dst, xt[:])
                    # x_hw rows: PE-transpose both c tiles, reduce, combine
                    for g8 in range(HC // 8):
                        hw8 = [cols.tile([128, 8], f32, name=f"hw8{s}{k}{g8}{t}",
                                         tag="hw8") for t in range(CT)]
                        for t in range(CT):
                            tr = ps_t.tile([128, 8, 128], f32,
                                           name=f"tr{s}{k}{g8}{t}", tag="tr")
                            for j in range(8):
                                nc.tensor.transpose(
                                    tr[:, j, :], xts[t][:, g8 * 8 + j, :],
                                    identity[:])
                            nc.vector.tensor_reduce(
                                out=hw8[t][:], in_=tr[:], axis=X, op=Alu.max)
                        nc.vector.tensor_tensor(
                            out=xhwT[s][:, k * HC + g8 * 8: k * HC + g8 * 8 + 8],
                            in0=hw8[0][:], in1=hw8[1][:], op=Alu.max)
                for t in range(CT):
                    nc.vector.tensor_reduce(
                        out=xcw[s][:, t * W:(t + 1) * W],
                        in_=xcwp[t][:].rearrange("p (k w) -> p w k", w=W),
                        axis=X, op=Alu.max)

            if "B" not in phases:
                return

            # ---------------- phase B: attention ----------------
            def psum_copy_to(dst, src_ps):
                nc.scalar.copy(dst, src_ps)

            def transpose_to(dst, src_sb, nblk, name):
                # src [128, nblk*128] -> dst [128, nblk*128] blockwise T
                for t in range(nblk):
                    tp = ps_mm.tile([128, 128], f32, name=f"tp{name}{t}",
                                    tag="mm")
                    nc.tensor.transpose(
                        tp[:], src_sb[:, t * 128:(t + 1) * 128], identity[:])
                    psum_copy_to(dst[:, t * 128:(t + 1) * 128], tp[:])

            def softmax(s, br, sim_ps, ncol, e_dst):
                rowmax = cols.tile([128, 1], f32, name=f"rm{s}{br}", tag="c1")
                nc.vector.tensor_reduce(out=rowmax[:], in_=sim_ps[:], axis=X,
                                        op=Alu.max)
                rmt = ps_ty.tile([1, 128], f32, name=f"rmt{s}{br}", tag="ty")
                nc.tensor.transpose(rmt[:], rowmax[:], identity[:])
                gmax = cols.tile([1, 1], f32, name=f"gm{s}{br}", tag="c0")
                nc.vector.tensor_reduce(out=gmax[:], in_=rmt[:], axis=X,
                                        op=Alu.max)
                ngmax = cols.tile([1, 1], f32, name=f"ngm{s}{br}", tag="c0")
                nc.scalar.mul(ngmax[:], gmax[:], -1.0)
                nm_ps = ps_ty.tile([128, 1], f32, name=f"nmp{s}{br}", tag="ty")
                nc.tensor.matmul(nm_ps[:], ones_r[:], ngmax[:])
                nmcol = cols.tile([128, 1], f32, name=f"nmc{s}{br}", tag="c1")
                psum_copy_to(nmcol[:], nm_ps[:])
                rowsum = cols.tile([128, 1], f32, name=f"rs{s}{br}", tag="c1")
                nc.scalar.activation(out=e_dst[:], in_=sim_ps[:], func=Act.Exp,
                                     bias=nmcol[:], scale=1.0,
                                     accum_out=rowsum[:])
                tot_ps = ps_ty.tile([1, 1], f32, name=f"tot{s}{br}", tag="ty")
                nc.tensor.matmul(tot_ps[:], rowsum[:], ones_c[:])
                invt = keep.tile([1, 1], f32, name=f"it{s}{br}",
                                 tag=f"it{s}{br}")
                nc.vector.reciprocal(invt[:], tot_ps[:])
                ic_ps = ps_ty.tile([128, 1], f32, name=f"icp{s}{br}", tag="ty")
                nc.tensor.matmul(ic_ps[:], ones_r[:], invt[:])
                iccol = keep.tile([128, 1], f32, name=f"icc{s}{br}",
                                  tag=f"icc{s}{br}")
                psum_copy_to(iccol[:], ic_ps[:])
                it1[(s, br)] = invt
                itc[(s, br)] = iccol

            scht = bmap("scht", [128, CT * H])  # [h, (t,c_loc)] -> x_ch^T
            scwt = bmap("scwt", [128, CT * W])  # [w, (t,c_loc)] -> x_cw^T

            for s in range(S):
                transpose_to(scht[s], xch[s], CT, f"sch{s}")
                transpose_to(scwt[s], xcw[s], CT, f"scw{s}")
                shp = ps_mm.tile([128, 128], f32, name=f"shp{s}", tag="mm")
                nc.tensor.transpose(shp[:], xhwT[s][:], identity[:])
                psum_copy_to(shw[s][:], shp[:])

                # --- branch 12: sim12[w,c] = sum_h Shw[h,w] Sch[c,h]
                sim12 = ps_mm.tile([128, C], f32, name=f"s12_{s}", tag="mm")
                nc.tensor.matmul(sim12[:], shw[s][:], scht[s][:])
                softmax(s, 12, sim12, C, e12[s])
                transpose_to(e12t[s], e12[s], CT, f"e12{s}")
                # y12T[w,h] = sum_c e12t[c,w]^T ... accumulate 2 c tiles
                y12p = ps_mm.tile([128, H], f32, name=f"y12p{s}", tag="mm")
                for t in range(CT):
                    nc.tensor.matmul(
                        y12p[:], e12t[s][:, t * W:(t + 1) * W],
                        xch[s][:, t * H:(t + 1) * H],
                        start=(t == 0), stop=(t == CT - 1))
                psum_copy_to(y12T[s][:], y12p[:])
                # y21[c,h] per c tile
                for t in range(CT):
                    y21p = ps_mm.tile([128, H], f32, name=f"y21p{s}{t}",
                                      tag="mm")
                    nc.tensor.matmul(y21p[:], e12[s][:, t * 128:(t + 1) * 128],
                                     xhwT[s][:])
                    psum_copy_to(ymt(s, 0, t), y21p[:])

                # --- branch 13: sim13[h,c] = sum_w Shw[h,w] Scw[c,w]
                sim13 = ps_mm.tile([128, C], f32, name=f"s13_{s}", tag="mm")
                nc.tensor.matmul(sim13[:], xhwT[s][:], scwt[s][:])
                softmax(s, 13, sim13, C, e13[s])
                transpose_to(e13t[s], e13[s], CT, f"e13{s}")
                y13p = ps_mm.tile([128, W], f32, name=f"y13p{s}", tag="mm")
                for t in range(CT):
                    nc.tensor.matmul(
                        y13p[:], e13t[s][:, t * H:(t + 1) * H],
                        xcw[s][:, t * W:(t + 1) * W],
                        start=(t == 0), stop=(t == CT - 1))
                psum_copy_to(y13[s][:], y13p[:])
                for t in range(CT):
                    y31p = ps_mm.tile([128, W], f32, name=f"y31p{s}{t}",
                                      tag="mm")
                    nc.tensor.matmul(y31p[:], e13[s][:, t * 128:(t + 1) * 128],
                                     shw[s][:])
                    psum_copy_to(ymt(s, 2, t), y31p[:])

                # --- branch 23: sim23[h,w] = sum_c Sch[c,h] Scw[c,w]
                sim23 = ps_mm.tile([128, W], f32, name=f"s23_{s}", tag="mm")
                for t in range(CT):
                    nc.tensor.matmul(
                        sim23[:], xch[s][:, t * H:(t + 1) * H],
                        xcw[s][:, t * W:(t + 1) * W],
                        start=(t == 0), stop=(t == CT - 1))
                softmax(s, 23, sim23, W, e23[s])
                transpose_to(e23t[s], e23[s], 1, f"e23{s}")
                for t in range(CT):
                    y23p = ps_mm.tile([128, H], f32, name=f"y23p{s}{t}",
                                      tag="mm")
                    nc.tensor.matmul(y23p[:], scwt[s][:, t * W:(t + 1) * W],
                                     e23t[s][:])
                    psum_copy_to(ymt(s, 1, t), y23p[:])
                    y32p = ps_mm.tile([128, W], f32, name=f"y32p{s}{t}",
                                      tag="mm")
                    nc.tensor.matmul(y32p[:], scht[s][:, t * H:(t + 1) * H],
                                     e23[s][:])
                    psum_copy_to(ymt(s, 3, t), y32p[:])

                # y12 = transpose(y12T)
                y12pp = ps_mm.tile([128, 128], f32, name=f"y12pp{s}", tag="mm")
                nc.tensor.transpose(y12pp[:], y12T[s][:], identity[:])
                psum_copy_to(y12[s][:], y12pp[:])

            if "C" not in phases:
                return

            # ---------------- phase C: BN partials ----------------
            ysq = gscr.tile([128, 128], f32, name="ysq", tag="ysq", bufs=2)
            for s in range(S):
                it2 = {}
                for br in (12, 13, 23):
                    t2 = keep.tile([128, 1], f32, name=f"it2_{s}{br}",
                                   tag=f"it2_{s}{br}")
                    nc.vector.tensor_tensor(out=t2[:], in0=itc[(s, br)][:],
                                            in1=itc[(s, br)][:], op=Alu.mult)
                    it2[br] = t2
                bnc_maps = [(0, 12), (1, 23), (2, 13), (3, 23)]
                for m, br in bnc_maps:
                    r2 = cols.tile([128, 2], f32, name=f"r{s}{m}", tag="c2")
                    nc.vector.tensor_reduce(
                        out=r2[:], in_=ym(s, m).rearrange("p (t h) -> p t h",
                                                          t=CT),
                        axis=X, op=Alu.add)
                    nc.vector.scalar_tensor_tensor(
                        out=bnp[:, m * 2:m * 2 + 2], in0=r2[:],
                        scalar=itc[(s, br)][:], in1=bnp[:, m * 2:m * 2 + 2],
                        op0=Alu.mult, op1=Alu.add)
                    for t in range(CT):
                        col = m * 2 + t
                        blk = ymt(s, m, t)
                        sq = cols.tile([128, 1], f32, name=f"sq{s}{m}{t}",
                                       tag="c1")
                        nc.scalar.activation(out=ysq[:], in_=blk,
                                             func=Act.Square, accum_out=sq[:])
                        nc.vector.scalar_tensor_tensor(
                            out=bnp[:, 8 + col:9 + col], in0=sq[:],
                            scalar=it2[br][:], in1=bnp[:, 8 + col:9 + col],
                            op0=Alu.mult, op1=Alu.add)
                # bn1 partials (partition 0, cols 16..19)
                for j, (ymap, br) in enumerate(((y12T[s], 12), (y13[s], 13))):
                    i1 = it1[(s, br)]
                    i2 = cols.tile([1, 1], f32, name=f"i2_{s}{j}", tag="c0")
                    nc.vector.tensor_tensor(out=i2[:], in0=i1[:], in1=i1[:],
                                            op=Alu.mult)
                    rs = cols.tile([128, 1], f32, name=f"rs1_{s}{j}", tag="c1")
                    nc.vector.tensor_reduce(out=rs[:], in_=ymap[:], axis=X,
                                            op=Alu.add)
                    tp = ps_ty.tile([1, 1], f32, name=f"t1_{s}{j}", tag="ty")
                    nc.tensor.matmul(tp[:], rs[:], ones_c[:])
                    nc.vector.scalar_tensor_tensor(
                        out=bnp[0:1, 16 + 2 * j:17 + 2 * j], in0=tp[:],
                        scalar=i1[:], in1=bnp[0:1, 16 + 2 * j:17 + 2 * j],
                        op0=Alu.mult, op1=Alu.add)
                    sqc = cols.tile([128, 1], f32, name=f"sqc{s}{j}", tag="c1")
                    nc.scalar.activation(out=ysq[:], in_=ymap[:],
                                         func=Act.Square, accum_out=sqc[:])
                    tp2 = ps_ty.tile([1, 1], f32, name=f"t2_{s}{j}", tag="ty")
                    nc.tensor.matmul(tp2[:], sqc[:], ones_c[:])
                    nc.vector.scalar_tensor_tensor(
                        out=bnp[0:1, 17 + 2 * j:18 + 2 * j], in0=tp2[:],
                        scalar=i2[:], in1=bnp[0:1, 17 + 2 * j:18 + 2 * j],
                        op0=Alu.mult, op1=Alu.add)

            # pre-collective: per-sample itc columns in the (m,t) layout
            itc8 = []
            for s in range(S):
                t8 = keep.tile([128, 8], f32, name=f"itc8_{s}",
                               tag=f"itc8_{s}")
                for m, br in enumerate((12, 23, 13, 23)):
                    nc.vector.tensor_copy(
                        t8[:, 2 * m:2 * m + 2],
                        itc[(s, br)][:].broadcast_to([128, 2]))
                itc8.append(t8)

            # ---------------- allreduce ----------------
            nc.sync.dma_start(ccin, bnp[:])
            if n_cores > 1:
                nc.gpsimd.collective_compute(
                    "AllReduce", Alu.add,
                    replica_groups=[list(range(n_cores))],
                    ins=[ccin], outs=[ccout])
            else:
                nc.sync.dma_start(ccout, ccin)
            bnpg = persist.tile([128, NP], f32)
            nc.sync.dma_start(bnpg[:], ccout)

            # ---------------- phase D: BN finalize + gates ----------------
            # bn1 scalar chain first so the A gates (and the afull broadcast
            # DMA) unblock phase E as early as possible
            sc1 = []
            sh1 = []
            for j in range(2):
                mu1 = cols.tile([1, 1], f32, name=f"mu1_{j}", tag="c0")
                nc.vector.tensor_scalar_mul(
                    mu1[:], bnpg[0:1, 16 + 2 * j:17 + 2 * j], 1.0 / n1)
                m21 = cols.tile([1, 1], f32, name=f"m21_{j}", tag="c0")
                nc.vector.tensor_tensor(out=m21[:], in0=mu1[:], in1=mu1[:],
                                        op=Alu.mult)
                v1 = cols.tile([1, 1], f32, name=f"v1_{j}", tag="c0")
                nc.vector.scalar_tensor_tensor(
                    out=v1[:], in0=bnpg[0:1, 17 + 2 * j:18 + 2 * j],
                    scalar=1.0 / n1, in1=m21[:], op0=Alu.mult,
                    op1=Alu.subtract)
                sd1 = cols.tile([1, 1], f32, name=f"sd1_{j}", tag="c0")
                nc.scalar.activation(out=sd1[:], in_=v1[:], func=Act.Sqrt,
                                     bias=eps_col[0:1, :])
                rst1 = cols.tile([1, 1], f32, name=f"rst1_{j}", tag="c0")
                nc.vector.reciprocal(rst1[:], sd1[:])
                sc = keep.tile([1, 1], f32, name=f"sc1_{j}", tag=f"sc1_{j}")
                nc.vector.tensor_tensor(out=sc[:], in0=rst1[:],
                                        in1=bn1w_sb[:], op=Alu.mult)
                sc1.append(sc)
                q1 = cols.tile([1, 1], f32, name=f"q1_{j}", tag="c0")
                nc.vector.tensor_tensor(out=q1[:], in0=mu1[:], in1=sc[:],
                                        op=Alu.mult)
                sh = keep.tile([1, 1], f32, name=f"sh1_{j}", tag=f"sh1_{j}")
                nc.vector.scalar_tensor_tensor(
                    out=sh[:], in0=q1[:], scalar=-1.0, in1=bn1b_sb[:],
                    op0=Alu.mult, op1=Alu.add)
                sh1.append(sh)

            for s in range(S):
                rhs4 = cols.tile([1, 4], f32, name=f"rhs4_{s}", tag="c4")
                for j, br in ((0, 12), (1, 13)):
                    nc.vector.tensor_tensor(out=rhs4[:, j:j + 1],
                                            in0=sc1[j][:],
                                            in1=it1[(s, br)][:], op=Alu.mult)
                    nc.vector.tensor_copy(rhs4[:, 2 + j:3 + j], sh1[j][:])
                cm_ps = ps_ty.tile([128, 4], f32, name=f"cmp{s}", tag="ty")
                nc.tensor.matmul(cm_ps[:], ones_r[:], rhs4[:])
                colmat = cols.tile([128, 4], f32, name=f"cm{s}", tag="c4b")
                psum_copy_to(colmat[:], cm_ps[:])
                g1 = gscr.tile([128, W], bf16, name=f"g12_{s}", tag="ga")
                g2 = gscr.tile([128, W], bf16, name=f"g13_{s}", tag="ga")
                for j, (ymap, g) in enumerate(((y12[s], g1), (y13[s], g2))):
                    nc.scalar.activation(out=g[:], in_=ymap[:],
                                         func=Act.Sigmoid,
                                         bias=colmat[:, 2 + j:3 + j],
                                         scale=colmat[:, j:j + 1])
                nc.vector.tensor_tensor(out=agate[s][:], in0=g1[:], in1=g2[:],
                                        op=Alu.mult)
                nc.scalar.dma_start(
                    adram[s].rearrange("(h w) -> h w", h=H), agate[s][:])

            # bnc finalize (batched over all 4 maps x 2 c-tiles)
            sm = persist.tile([128, 8], f32, name="mu8")
            nc.vector.tensor_scalar_mul(sm[:], bnpg[:, 0:8], 1.0 / ncn)
            m2 = persist.tile([128, 8], f32, name="m28")
            nc.scalar.activation(out=m2[:], in_=bnpg[:, 0:8], func=Act.Square,
                                 scale=1.0 / ncn)
            var8 = persist.tile([128, 8], f32, name="var8")
            nc.vector.scalar_tensor_tensor(
                out=var8[:], in0=bnpg[:, 8:16], scalar=1.0 / ncn, in1=m2[:],
                op0=Alu.mult, op1=Alu.subtract)
            sd8 = persist.tile([128, 8], f32, name="sd8")
            nc.scalar.activation(out=sd8[:], in_=var8[:], func=Act.Sqrt,
                                 bias=eps_col[:])
            rstd8 = persist.tile([128, 8], f32, name="rstd8")
            nc.vector.reciprocal(rstd8[:], sd8[:])
            scale8 = persist.tile([128, 8], f32, name="scale8")
            nc.vector.tensor_tensor(out=scale8[:], in0=rstd8[:], in1=wc8[:],
                                    op=Alu.mult)
            q8 = persist.tile([128, 8], f32, name="q8")
            nc.vector.tensor_tensor(out=q8[:], in0=sm[:], in1=scale8[:],
                                    op=Alu.mult)
            shift8 = persist.tile([128, 8], f32, name="shift8")
            nc.vector.scalar_tensor_tensor(
                out=shift8[:], in0=q8[:], scalar=-1.0, in1=bc8[:],
                op0=Alu.mult, op1=Alu.add)

            bgate = smap("bgate", [128, CT * H], dtype=bf16)
            cgate = smap("cgate", [128, CT * W], dtype=bf16)
            for s in range(S):
                scc8 = cols.tile([128, 8], f32, name=f"scc8_{s}", tag="c8")
                nc.vector.tensor_tensor(out=scc8[:], in0=scale8[:],
                                        in1=itc8[s][:], op=Alu.mult)
                arg = gscr.tile([128, 8, 128], f32, name=f"arg{s}", tag="arg",
                                bufs=1)
                nc.vector.tensor_tensor(
                    out=arg[:],
                    in0=Y8[s][:].rearrange("p (m h) -> p m h", m=8),
                    in1=scc8[:].unsqueeze(2).broadcast_to([128, 8, 128]),
                    op=Alu.mult)
                nc.vector.tensor_tensor(
                    out=arg[:], in0=arg[:],
                    in1=shift8[:].unsqueeze(2).broadcast_to([128, 8, 128]),
                    op=Alu.add)
                gh = gscr.tile([128, 8, 128], bf16, name=f"gh{s}", tag="gh",
                               bufs=2)
                nc.scalar.activation(out=gh[:], in_=arg[:], func=Act.Sigmoid)
                nc.vector.tensor_tensor(
                    out=bgate[s][:].rearrange("p (t h) -> p t h", t=CT),
                    in0=gh[:, 0:2, :], in1=gh[:, 2:4, :], op=Alu.mult)
                nc.vector.tensor_tensor(
                    out=cgate[s][:].rearrange("p (t w) -> p t w", t=CT),
                    in0=gh[:, 4:6, :], in1=gh[:, 6:8, :], op=Alu.mult)

        # ---------------- phase E: apply (partition = channel) ----------------
        if "E" not in phases:
            return
        e_stack = contextlib.ExitStack()
        with e_stack:
            pme = e_stack.enter_context(tc.tile_pool(name="pme", bufs=2))
            poe = e_stack.enter_context(tc.tile_pool(name="poe", bufs=2))
            paf = e_stack.enter_context(tc.tile_pool(name="paf", bufs=1))

            for s in range(S):
                # replicate A = g12*g13 [h,w] across the 128 c partitions
                # via a stride-0 partition-broadcast DMA load
                afull = paf.tile([128, H, W], bf16, name=f"af{s}", tag="af")
                nc.sync.dma_start(
                    afull[:], adram[s].rearrange("(h w) -> h w", h=H)
                    .unsqueeze(0).broadcast_to([128, H, W]))

                for t in range(CT):
                    bsl_all = bgate[s][:, t * H:(t + 1) * H]
                    csl = cgate[s][:, t * W:(t + 1) * W] \
                        .unsqueeze(1).broadcast_to([128, HC, W])
                    for k in range(NCH):
                        xsrc = xres[s][t][:, k * HC:(k + 1) * HC, :]
                        m = pme.tile([128, HC, W], bf16, name=f"me{s}{t}{k}",
                                     tag="me")
                        bsl = bsl_all[:, k * HC:(k + 1) * HC] \
                            .unsqueeze(2).broadcast_to([128, HC, W])
                        nc.vector.tensor_tensor(out=m[:], in0=bsl, in1=csl,
                                                op=Alu.mult)
                        nc.vector.tensor_tensor(
                            out=m[:], in0=m[:],
                            in1=afull[:, k * HC:(k + 1) * HC, :], op=Alu.mult)
                        o = poe.tile([128, HC, W], bf16, name=f"oe{s}{t}{k}",
                                     tag="oe")
                        nc.vector.scalar_tensor_tensor(
                            out=o[:], in0=m[:], scalar=1.0, in1=xsrc,
                            op0=Alu.add, op1=Alu.mult)
                        nc.scalar.dma_start(
                            outy[s, t * 128:(t + 1) * 128,
                                 k * HC:(k + 1) * HC, :], o[:])


_NC_CACHE = {}
LAST_RESULT = None


def _get_nc(n_cores: int, sync_start: bool = False, phases: str = "ABCDE"):
    key = (n_cores, sync_start, phases)
    if key not in _NC_CACHE:
        _NC_CACHE[key] = build_bass(n_cores, sync_start, phases)
    return _NC_CACHE[key]


def kernel(**inputs) -> np.ndarray:
    from concourse.bass_utils import run_bass_kernel_spmd

    x = np.ascontiguousarray(inputs["x"], dtype=np.float32)
    bn1_w = np.ascontiguousarray(inputs["bn1_w"], dtype=np.float32)
    bn1_b = np.ascontiguousarray(inputs["bn1_b"], dtype=np.float32)
    bnc_w = np.ascontiguousarray(inputs["bnc_w"], dtype=np.float32)
    bnc_b = np.ascontiguousarray(inputs["bnc_b"], dtype=np.float32)
    B = x.shape[0]
    assert B == NCORES * S, (B, NCORES, S)

    nc = _get_nc(NCORES)
    in_maps = []
    for i in range(NCORES):
        in_maps.append({
            "xs": np.ascontiguousarray(x[i * S:(i + 1) * S]),
            "bn1_w": bn1_w, "bn1_b": bn1_b,
            "bnc_w": bnc_w, "bnc_b": bnc_b,
        })
    res = run_bass_kernel_spmd(nc, in_maps, core_ids=list(range(NCORES)))
    global LAST_RESULT
    LAST_RESULT = res
    out = np.concatenate(
        [np.asarray(res.results[i]["outy"]).astype(np.float32)
         for i in range(NCORES)], axis=0)
    return out
